# revision 12
# baseline (speedup 1.0000x reference)
"""Multi-head attention (B=4, T=2048, D=768, H=12) on 8 NeuronCores.

Sharding: core c handles batch b = c//2 and head-group g = c%2 (heads
6g..6g+5).  Each core computes its 6 heads' attention and a partial
output projection (contraction over its 384 local dims of w_proj).  The
host sums the two partials per batch and adds the bias terms.

Device-side formulation (per core):
  xT   [768, 2048]  (host pre-transposes x[b])
  qT   = Wq_loc.T @ xT   [384, 2048]   scaled by log2e/sqrt(hd), +bias
  kT   = Wk_loc.T @ xT   [384, 2048]   (+bias)
  v    = x @ Wv_loc      [2048, 384]   (bf16; v-bias folded on host)
  S^T  = kT_h.T @ qT_h   [kpos, q] per (head, kt-tile), base-2 domain
  P^T  = exp2(S^T) = Exp activation with scale=ln2 (bf16)
  O    accumulated in PSUM [q, d] via stationary-P matmuls; softmax
       denominators via ones-column matmuls into a second PSUM bank
  oT   = PE-transpose of the normalized O (bf16, via identity matmul)
  y    = O_loc @ Wp_loc partials shipped bf16 as two tensors; the host
         sums partials from both cores per batch + biases

Schedule: a single global stream of 192 steps (6 heads x 2 q-halves x
16 kt tiles), paced by the ACT exp stream (~1us/step).  Per step the PE
runs the next step's S matmuls, this step's AV/denominator matmuls, and
"filler" work (qkv projections, V tiles, transposes, output projection)
assigned by a small deadline-driven scheduler so PE load stays under
the exp pace everywhere.  At q-half boundaries the AV emissions are
deferred two steps so the in-order PE never head-blocks on the
normalize/memset chain; the exp stream free-runs.  Evacuations are
spread across DVE and Pool.  A subset of exp tiles is computed on
DVE/Pool with a bit-trick exp2 (magic-constant round, quadratic 2^f,
exponent-field add) to keep ACT below the PE roofline.
"""

import numpy as np

EMBED = 768
HEADS = 12
HD = 64
SCALE = HD ** -0.5
LOG2E = 1.4426950408889634
LN2 = 0.6931471805599453
MAGIC = 12582912.0  # 1.5 * 2^23
EXPC2, EXPC1, EXPC0 = 0.23842249585793798, -0.7034364107920545, 1.000442696284017
B, T = 4, 2048
NCORES = 8
HPC = 6            # heads per core
DL = HPC * HD      # 384 local model dims per core

NDT = EMBED // 128   # 6 contraction tiles over embed dim
NKT = T // 128       # 16 key-position tiles
NQT = T // 128       # 16 q tiles
QH = 2               # q halves of 1024
QHW = T // QH        # 1024
QT = QHW // 128      # 8 q-tiles per half
NST = QH * NKT       # 32 steps per head
NSTEP = HPC * NST    # 192 global steps

# global steps on which the exp tile is computed on DVE/Pool instead of ACT
DVE_EXP_STEPS = frozenset()

_prog_cache = {}


def _schedule(items, budgets):
    """EDF-greedy: place each (release, deadline, cyc, fn) item at the
    earliest step with room; overflow lands at the deadline."""
    nsteps = len(budgets)
    per_step = [[] for _ in range(nsteps)]
    load = [0] * nsteps
    for idx, it in sorted(enumerate(items),
                          key=lambda p: (p[1][1], p[1][0], p[0])):
        release, deadline, cyc, fn = it
        deadline = min(deadline, nsteps - 1)
        for s in range(release, deadline + 1):
            if load[s] + cyc <= budgets[s]:
                per_step[s].append(fn)
                load[s] += cyc
                break
        else:
            per_step[deadline].append(fn)
            load[deadline] += cyc
    return per_step


def _build_program(repeat=1):
    import concourse.bass as bass
    import concourse.mybir as mybir
    import concourse.tile as tile
    from concourse import bacc

    f32 = mybir.dt.float32
    f32r = mybir.dt.float32r
    bf16 = mybir.dt.bfloat16
    i32 = mybir.dt.int32
    i16 = mybir.dt.int16
    ACT_EXP = mybir.ActivationFunctionType.Exp
    ACT_COPY = mybir.ActivationFunctionType.Copy
    QSC = float(SCALE * LOG2E)

    nc = bacc.Bacc()

    xt_d = nc.dram_tensor("xt", [EMBED, T], f32r, kind="ExternalInput")
    wq_d = nc.dram_tensor("wq", [EMBED, DL], f32r, kind="ExternalInput")
    wk_d = nc.dram_tensor("wk", [EMBED, DL], f32r, kind="ExternalInput")
    wv_d = nc.dram_tensor("wv", [EMBED, DL], bf16, kind="ExternalInput")
    bqs_d = nc.dram_tensor("bqs", [DL], f32, kind="ExternalInput")
    bk_d = nc.dram_tensor("bk", [DL], f32, kind="ExternalInput")
    wp_d = nc.dram_tensor("wp", [DL, EMBED], bf16, kind="ExternalInput")
    aux_d = nc.dram_tensor("aux", [128, 129], f32, kind="ExternalInput")
    y_d = nc.dram_tensor("y", [T, EMBED], bf16, kind="ExternalOutput")
    y2_d = nc.dram_tensor("y2", [T, EMBED], bf16, kind="ExternalOutput")

    with tile.TileContext(nc) as tc:
      for _rep in range(repeat):
        with tc.tile_pool(name="pers", bufs=1) as pers, \
             tc.tile_pool(name="qk", bufs=3) as qk_pool, \
             tc.tile_pool(name="pT", bufs=8) as pT_pool, \
             tc.tile_pool(name="rcp", bufs=2) as rcp_pool, \
             tc.tile_pool(name="ysh", bufs=4) as ysh_pool, \
             tc.tile_pool(name="pss", bufs=2, space="PSUM") as pss_pool, \
             tc.tile_pool(name="po", bufs=1, space="PSUM") as po_pool, \
             tc.tile_pool(name="pd", bufs=1, space="PSUM") as pd_pool, \
             tc.tile_pool(name="ps", bufs=2, space="PSUM") as ps_pool:
            xt_sb = pers.tile([128, 4, NDT, 512], f32r, name="xt_sb")
            xtb_sb = pers.tile([128, 4, NDT, 512], bf16, name="xtb_sb")
            wq_sb = pers.tile([128, NDT, DL], f32r, name="wq_sb")
            wk_sb = pers.tile([128, NDT, DL], f32r, name="wk_sb")
            wv_sb = pers.tile([128, NDT, DL], bf16, name="wv_sb")
            wp_sb = pers.tile([128, 3, EMBED], bf16, name="wp_sb")
            v_sb = pers.tile([128, NKT, HPC, HD], bf16, name="v_sb")
            oT_sb = pers.tile([128, 3, T], bf16, name="oT_sb")
            pack_sb = pers.tile([128, NQT, 128], bf16, name="pack_sb")
            eye_sb = pers.tile([128, 128], bf16, name="eye_sb")
            ones_sb = pers.tile([128, 1], bf16, name="ones_sb")
            auxf_sb = pers.tile([128, 129], f32, name="auxf_sb")
            bqs_sb = pers.tile([128, 3], f32, name="bqs_sb")
            bk_sb = pers.tile([128, 3], f32, name="bk_sb")
            warm_sb = pers.tile([128, 512], f32r, name="warm_sb")

            po_ps = po_pool.tile([128, QT, HD], f32, name="po_ps")
            pd_ps = pd_pool.tile([128, QH, QT], f32, name="pd_ps")

            # ---------------- startup DMAs ----------------
            wq_r = wq_d.ap().rearrange("(n p) m -> p n m", p=128)
            wk_r = wk_d.ap().rearrange("(n p) m -> p n m", p=128)
            wv_r = wv_d.ap().rearrange("(n p) m -> p n m", p=128)

            def dma_xt(ch):
                for dh in range(2):
                    nc.sync.dma_start(
                        out=xt_sb[:, ch, 3 * dh:3 * dh + 3, :],
                        in_=xt_d.ap()[bass.ds(384 * dh, 384), bass.ts(ch, 512)]
                        .rearrange("(n p) m -> p n m", p=128),
                    )

            def copy_xtb(ch):
                for dh in range(2):
                    nc.gpsimd.tensor_copy(
                        out=xtb_sb[:, ch, 3 * dh:3 * dh + 3, :],
                        in_=xt_sb[:, ch, 3 * dh:3 * dh + 3, :],
                    )

            # startup-critical order: pair-0 weight columns, x chunk 0/1,
            # wv (V tiles start early), then the rest
            nc.sync.dma_start(out=wq_sb[:, :, 0:128], in_=wq_r[:, :, 0:128])
            nc.sync.dma_start(out=wk_sb[:, :, 0:128], in_=wk_r[:, :, 0:128])
            nc.sync.dma_start(out=bqs_sb, in_=bqs_d.ap().rearrange("(n p) -> p n", p=128))
            nc.sync.dma_start(out=bk_sb, in_=bk_d.ap().rearrange("(n p) -> p n", p=128))
            dma_xt(0)
            dma_xt(1)
            nc.sync.dma_start(out=wv_sb, in_=wv_r)
            dma_xt(2)
            dma_xt(3)
            nc.sync.dma_start(out=wq_sb[:, :, 128:384], in_=wq_r[:, :, 128:384])
            nc.sync.dma_start(out=wk_sb[:, :, 128:384], in_=wk_r[:, :, 128:384])
            nc.sync.dma_start(out=auxf_sb, in_=aux_d.ap())
            nc.gpsimd.dma_start(
                out=wp_sb, in_=wp_d.ap().rearrange("(n p) m -> p n m", p=128))

            # PE warm-up while the first input DMAs stream (p-state ramp)
            nc.vector.memset(warm_sb.bitcast(f32), 0.0)
            for _wi in range(18):
                psw = ps_pool.tile([128, 512], f32, name="psw", tag="ps")
                nc.tensor.matmul(psw, warm_sb[0:2, 0:128], warm_sb[0:2, :],
                                 start=True, stop=True)

            copy_xtb(0)
            copy_xtb(1)
            nc.vector.tensor_copy(out=eye_sb, in_=auxf_sb[:, 0:128])
            nc.vector.tensor_copy(out=ones_sb, in_=auxf_sb[:, 128:129])

            # ---------------- emitters ----------------
            qk_tiles = {}

            def new_qk(hp):
                if hp in qk_tiles:
                    return
                qTp = qk_pool.tile([128, T], f32r, name="qTp", tag="qT")
                kTp = qk_pool.tile([128, T], f32r, name="kTp", tag="kT")
                qk_tiles[hp] = (qTp, kTp)

            def emit_qk(hp, isq, ch, half):
                """One contraction-half of a qk projection chunk: 3 matmuls
                into a fresh ps tile; half 0 writes qT/kT (with bias), half 1
                accumulates on top."""
                new_qk(hp)
                qTp, kTp = qk_tiles[hp]
                ps = ps_pool.tile([128, 512], f32, name="psqk", tag="ps")
                w_sb = wq_sb if isq else wk_sb
                for dt in range(3 * half, 3 * half + 3):
                    nc.tensor.matmul(
                        ps,
                        w_sb[:, dt, bass.ts(hp, 128)],
                        xt_sb[:, ch, dt, :],
                        start=(dt == 3 * half),
                        stop=(dt == 3 * half + 2),
                    )
                csl = bass.ts(ch, 512)
                if half == 0:
                    if isq:
                        nc.vector.tensor_scalar(
                            out=qTp[:, csl], in0=ps,
                            scalar1=bqs_sb[:, hp:hp + 1],
                            scalar2=QSC,
                            op0=mybir.AluOpType.add,
                            op1=mybir.AluOpType.mult,
                        )
                    else:
                        nc.vector.tensor_scalar_add(
                            out=kTp[:, csl], in0=ps,
                            scalar1=bk_sb[:, hp:hp + 1],
                        )
                else:
                    if isq:
                        nc.vector.scalar_tensor_tensor(
                            out=qTp[:, csl], in0=ps,
                            scalar=QSC, in1=qTp[:, csl],
                            op0=mybir.AluOpType.mult,
                            op1=mybir.AluOpType.add,
                        )
                    else:
                        nc.vector.tensor_add(
                            out=kTp[:, csl], in0=kTp[:, csl], in1=ps,
                        )

            def emit_v_half(kt, half):
                psv = ps_pool.tile([128, DL // 2], f32, name="psv", tag="ps")
                vsl = bass.ds(half * (DL // 2), DL // 2)
                for dt in range(NDT):
                    nc.tensor.matmul(
                        psv,
                        xtb_sb[:, kt // 4, dt, bass.ds((kt % 4) * 128, 128)],
                        wv_sb[:, dt, vsl],
                        start=(dt == 0), stop=(dt == NDT - 1),
                    )
                nc.vector.tensor_copy(
                    out=v_sb[:, kt, 3 * half:3 * half + 3, :],
                    in_=psv.rearrange("p (h d) -> p h d", h=3),
                )

            def transpose_qt(hp, qtg, evac="pool"):
                oTps = ps_pool.tile([128, 128], bf16, name="oTps", tag="ps")
                nc.tensor.matmul(
                    oTps, pack_sb[:, qtg, :], eye_sb, is_transpose=True,
                )
                dst = oT_sb[:, hp, bass.ts(qtg, 128)]
                if evac == "act":
                    nc.scalar.copy(out=dst, in_=oTps)
                else:
                    nc.vector.tensor_copy(out=dst, in_=oTps)

            # ---------------- attention steps ----------------
            SEQ = [(h, qh, kt) for h in range(HPC) for qh in range(QH)
                   for kt in range(NKT)]

            pend_S = {}

            def emit_S(h, qh, kt):
                hp, off = h // 2, (h % 2) * 64
                qTp, kTp = qk_tiles[hp]
                pss = pss_pool.tile([128, QHW], f32, name="pss", tag="pss")
                for c2 in range(QHW // 512):
                    nc.tensor.matmul(
                        pss[:, bass.ts(c2, 512)],
                        kTp[off:off + 64, bass.ts(kt, 128)],
                        qTp[off:off + 64, bass.ds(qh * QHW + c2 * 512, 512)],
                        start=True, stop=True,
                    )
                pend_S[(h, qh, kt)] = pss

            def emit_exp(h, qh, kt):
                """exp2 of a pending S tile on ACT (scale=ln2 folds the
                base-2 pre-scaling back into e^x)."""
                pss = pend_S.pop((h, qh, kt))
                pT = pT_pool.tile([128, QHW], bf16, name="pT", tag="pT")
                nc.scalar.activation(out=pT, in_=pss, func=ACT_EXP, scale=LN2)
                return pT

            def emit_av(h, qh, kt, pT):
                for qt in range(QT):
                    nc.tensor.matmul(
                        pd_ps[:, qh, qt:qt + 1],
                        pT[:, bass.ts(qt, 128)],
                        ones_sb,
                        start=(kt == 0 and qt == 0),
                        stop=(kt == NKT - 1 and qt == QT - 1),
                    )
                    nc.tensor.matmul(
                        po_ps[:, qt, :],
                        pT[:, bass.ts(qt, 128)],
                        v_sb[:, kt, h, :],
                        start=(kt == 0 and qt == 0),
                        stop=(kt == NKT - 1 and qt == QT - 1),
                    )

            def normalize_qh(h, qh, final=False):
                """softmax normalization into the pack tile + the full-range
                memsets that order the next q-half's start=True matmuls
                behind every normalize read."""
                off2 = (h % 2) * 64
                rc = rcp_pool.tile([128, QT], f32, name="rc", tag="rc")
                nc.vector.reciprocal(out=rc, in_=pd_ps[:, qh, :])
                for qt in range(QT):
                    dst = pack_sb[:, qh * QT + qt, off2:off2 + 64]
                    if final and qt % 2 == 0:
                        nc.scalar.activation(
                            out=dst, in_=po_ps[:, qt, :],
                            func=ACT_COPY, scale=rc[:, qt:qt + 1],
                        )
                    else:
                        nc.vector.tensor_scalar_mul(
                            out=dst, in0=po_ps[:, qt, :],
                            scalar1=rc[:, qt:qt + 1],
                        )
                if not final:
                    nc.vector.memset(po_ps[:, :, :], 0.0)
                    nc.vector.memset(pd_ps[:, :, :], 0.0)

            # ---------------- output projection ----------------
            ysh01 = {}

            def proj01_nh(qtg, nh):
                if qtg not in ysh01:
                    ysh01[qtg] = ysh_pool.tile(
                        [128, EMBED], bf16, name="ysh", tag="ysh")
                ysh = ysh01[qtg]
                psy = ps_pool.tile([128, 384], f32, name="psy", tag="ps")
                for dt in range(2):
                    nc.tensor.matmul(
                        psy,
                        oT_sb[:, dt, bass.ts(qtg, 128)],
                        wp_sb[:, dt, bass.ts(nh, 384)],
                        start=(dt == 0), stop=(dt == 1),
                    )
                nc.vector.tensor_copy(out=ysh[:, bass.ts(nh, 384)], in_=psy)
                if nh == 1:
                    nc.sync.dma_start(
                        out=y_d.ap()[bass.ts(qtg, 128), :],
                        in_=ysh01.pop(qtg),
                    )

            def ship2(qtg, evac="pool", deep=False):
                ysh = ysh_pool.tile([128, EMBED], bf16, name="ysh2", tag="ysh")
                for nh in range(2):
                    if deep:
                        pool_t = pss_pool.tile([128, QHW], f32, name="psy2d",
                                               tag="pss")
                        psy2 = pool_t[:, 0:384]
                    else:
                        psy2 = ps_pool.tile([128, 384], f32, name="psy2",
                                            tag="ps")
                    nc.tensor.matmul(
                        psy2,
                        oT_sb[:, 2, bass.ts(qtg, 128)],
                        wp_sb[:, 2, bass.ts(nh, 384)],
                        start=True, stop=True,
                    )
                    dst = ysh[:, bass.ts(nh, 384)]
                    if evac == "act":
                        nc.scalar.copy(out=dst, in_=psy2)
                    else:
                        nc.vector.tensor_copy(out=dst, in_=psy2)
                nc.sync.dma_start(out=y2_d.ap()[bass.ts(qtg, 128), :], in_=ysh)

            # ---------------- filler schedule ----------------
            # budgets in PE cycles per step on top of the S+AV base
            budgets = []
            for i, (h, qh, kt) in enumerate(SEQ):
                b = 850
                if kt in (0, 1):
                    b += 520      # AVs deferred out of this step
                elif kt in (2, 3):
                    b -= 520      # deferred AVs flushed here
                budgets.append(b)

            items = []

            def add(release, deadline, cyc, fn):
                items.append((release, deadline, cyc, fn))

            qk_cyc, v_cyc, tr_cyc, p01_cyc, s2_cyc = 1536, 1152, 128, 768, 768

            # V half-0 tiles kt 4..15 (0-3 emitted pre-stream)
            vrel = lambda kt: (0 if kt < 4 else 2 if kt < 8 else
                               6 if kt < 12 else 10)
            for kt in range(4, NKT):
                add(vrel(kt), kt, v_cyc,
                    lambda kt=kt: emit_v_half(kt, 0))
            # xtb shadow copies for chunks 2,3 (Pool; zero PE cost)
            add(3, 4, 0, lambda: copy_xtb(2))
            add(7, 8, 0, lambda: copy_xtb(3))
            # pair-0 remaining q/k chunks.  S(0,0,kt) is emitted at step
            # kt-2, so k-chunk c is needed by step 4c-3; q chunks 2,3 feed
            # q-half 1 whose first S is emitted at step 14.
            for half in range(2):
                add(4, 5, qk_cyc, lambda h=half: emit_qk(0, 0, 2, h))
                add(8, 9, qk_cyc, lambda h=half: emit_qk(0, 0, 3, h))
                add(4, 13, qk_cyc, lambda h=half: emit_qk(0, 1, 2, h))
                add(8, 13, qk_cyc, lambda h=half: emit_qk(0, 1, 3, h))
            # pair-1 qk (heads 2,3 start at step 64)
            for ch in range(4):
                for half in range(2):
                    add(14, 61 + 4 * ch, qk_cyc,
                        lambda c=ch, h=half: emit_qk(1, 0, c, h))
                    add(14, 61 if ch < 2 else 76, qk_cyc,
                        lambda c=ch, h=half: emit_qk(1, 1, c, h))
            # V half-1 tiles (first consumer: head 3 at step 96)
            for kt in range(NKT):
                add(12, 94 + kt, v_cyc, lambda kt=kt: emit_v_half(kt, 1))
            # pair-2 qk (heads 4,5 start at step 128)
            for ch in range(4):
                for half in range(2):
                    add(40, 125 + 4 * ch, qk_cyc,
                        lambda c=ch, h=half: emit_qk(2, 0, c, h))
                    add(40, 125 if ch < 2 else 140, qk_cyc,
                        lambda c=ch, h=half: emit_qk(2, 1, c, h))
            # pair-0 transposes: pack ready after head 1 (step 63)
            for qtg in range(NQT):
                add(65, 120, tr_cyc, lambda q=qtg: transpose_qt(0, q))
            # pair-1 transposes: pack ready after head 3 (step 127)
            for qtg in range(NQT):
                add(129, 133 + qtg, tr_cyc, lambda q=qtg: transpose_qt(1, q))
            # output projection dt<2 partials (need pair-0/1 transposes + wp)
            for qtg in range(NQT):
                for nh in range(2):
                    add(134 + qtg, 186, p01_cyc,
                        lambda q=qtg, n=nh: proj01_nh(q, n))
            # pair-2 transposes + dt=2 ships for q-half 0 (pack(qh0) ready
            # after head-5 qh0 = step 175); coupled so the ship's oT read is
            # always emitted after its transpose
            def tr_ship2(qtg):
                transpose_qt(2, qtg)
                ship2(qtg)

            for qtg in range(QT):
                add(177, 184 + qtg, tr_cyc + s2_cyc,
                    lambda q=qtg: tr_ship2(q))

            per_step = _schedule(items, budgets)

            # ---------------- pre-stream ----------------
            new_qk(0)
            for ch in range(2):
                emit_qk(0, 1, ch, 0)
                emit_qk(0, 1, ch, 1)
                emit_qk(0, 0, ch, 0)
                emit_qk(0, 0, ch, 1)
            for kt in range(4):
                emit_v_half(kt, 0)
            emit_S(0, 0, 0)
            emit_S(0, 0, 1)

            # ---------------- main stream ----------------
            deferred = []
            for i, (h, qh, kt) in enumerate(SEQ):
                for fn in per_step[i]:
                    fn()
                pT = emit_exp(h, qh, kt)
                if kt in (0, 1) and i >= 2:
                    # q-half boundary: hold AVs so the in-order PE never
                    # head-blocks on the normalize/memset chain
                    deferred.append((h, qh, kt, pT))
                else:
                    if deferred:
                        dh, dqh, dkt, dpT = deferred.pop(0)
                        emit_av(dh, dqh, dkt, dpT)
                    emit_av(h, qh, kt, pT)
                if kt == NKT - 1:
                    while deferred:
                        dh, dqh, dkt, dpT = deferred.pop(0)
                        emit_av(dh, dqh, dkt, dpT)
                    normalize_qh(h, qh, final=(i == NSTEP - 1))
                # next step's S matmuls (the step after next: its pss slot
                # was freed by this step's exp)
                if i + 2 < NSTEP:
                    nh_, nqh_, nkt_ = SEQ[i + 2]
                    if (nh_ % 2, nqh_, nkt_) == (0, 0, 0) and nh_ // 2 > h // 2:
                        new_qk(nh_ // 2)
                    emit_S(nh_, nqh_, nkt_)

            # ---------------- tail ----------------
            for qtg in range(QT, NQT):
                transpose_qt(2, qtg, evac=("act" if qtg % 2 == 0 else "dve"))
                ship2(qtg, evac=("dve" if qtg % 2 == 0 else "act"), deep=True)

    nc.finalize()
    return nc


def _shard_inputs(x, w_qkv, b_qkv, w_proj):
    import ml_dtypes

    aux = np.zeros((128, 129), dtype=np.float32)
    aux[:, 0:128] = np.eye(128, dtype=np.float32)
    aux[:, 128] = 1.0
    in_maps = []
    for c in range(NCORES):
        b, g = c // 2, c % 2
        sl = slice(DL * g, DL * g + DL)
        in_maps.append({
            "xt": np.ascontiguousarray(x[b].T),
            "wq": np.ascontiguousarray(w_qkv[:, sl]),
            "wk": np.ascontiguousarray(w_qkv[:, EMBED:][:, sl]),
            "wv": np.ascontiguousarray(w_qkv[:, 2 * EMBED:][:, sl]).astype(ml_dtypes.bfloat16),
            "bqs": np.ascontiguousarray(b_qkv[sl]),
            "bk": np.ascontiguousarray(b_qkv[EMBED:][sl]),
            "wp": np.ascontiguousarray(w_proj[sl, :]).astype(ml_dtypes.bfloat16),
            "aux": aux,
        })
    return in_maps


def kernel(x, w_qkv, b_qkv, w_proj, b_proj, _profile=False, _repeat=1):
    from concourse.bass_utils import run_bass_kernel_spmd

    x = np.asarray(x, dtype=np.float32)
    w_qkv = np.asarray(w_qkv, dtype=np.float32)
    b_qkv = np.asarray(b_qkv, dtype=np.float32)
    w_proj = np.asarray(w_proj, dtype=np.float32)
    b_proj = np.asarray(b_proj, dtype=np.float32)

    if _repeat not in _prog_cache:
        _prog_cache[_repeat] = _build_program(_repeat)
    nc = _prog_cache[_repeat]

    in_maps = _shard_inputs(x, w_qkv, b_qkv, w_proj)
    res = run_bass_kernel_spmd(
        nc, in_maps, list(range(NCORES)), trace=_profile,
    )

    # host-side gather: sum the dt<2 / dt=2 partials of the two head-group
    # cores per batch and add the bias row (v-bias folded through w_proj,
    # plus b_proj itself)
    bias_row = b_qkv[2 * EMBED:] @ w_proj + b_proj
    y = np.empty((B, T, EMBED), dtype=np.float32)
    for b in range(B):
        acc = np.broadcast_to(bias_row.astype(np.float32), (T, EMBED)).copy()
        for c in (2 * b, 2 * b + 1):
            acc += np.asarray(res.results[c]["y"], dtype=np.float32)
            acc += np.asarray(res.results[c]["y2"], dtype=np.float32)
        y[b] = acc
    if _profile:
        return y, res
    return y


# revision 19
# speedup vs baseline: 1.0061x; 1.0061x over previous
"""Multi-head attention (B=4, T=2048, D=768, H=12) on 8 NeuronCores.

Sharding: core c handles batch b = c//2 and head-group g = c%2 (heads
6g..6g+5).  Each core computes its 6 heads' attention and a partial
output projection (contraction over its 384 local dims of w_proj).  The
host sums the two partials per batch and adds the bias terms.

Device-side formulation (per core):
  xT   [768, 2048]  (host pre-transposes x[b])
  qT   = Wq_loc.T @ xT   [384, 2048]   scaled by log2e/sqrt(hd), +bias
  kT   = Wk_loc.T @ xT   [384, 2048]   (+bias)
  v    = x @ Wv_loc      [2048, 384]   (bf16; v-bias folded on host)
  S^T  = kT_h.T @ qT_h   [kpos, q] per (head, kt-tile), base-2 domain
  P^T  = exp2(S^T) = Exp activation with scale=ln2 (bf16)
  O    accumulated in PSUM [q, d] via stationary-P matmuls; softmax
       denominators via ones-column matmuls into a second PSUM bank
  oT   = PE-transpose of the normalized O (bf16, via identity matmul)
  y    = O_loc @ Wp_loc partials shipped bf16 as two tensors; the host
         sums partials from both cores per batch + biases

Schedule: a single global stream of 192 steps (6 heads x 2 q-halves x
16 kt tiles), paced by the ACT exp stream (~1us/step).  Per step the PE
runs the next step's S matmuls, this step's AV/denominator matmuls, and
"filler" work (qkv projections, V tiles, transposes, output projection)
assigned by a small deadline-driven scheduler so PE load stays under
the exp pace everywhere.  At q-half boundaries the AV emissions are
deferred two steps so the in-order PE never head-blocks on the
normalize/memset chain; the exp stream free-runs.  Evacuations are
spread across DVE and Pool.  A subset of exp tiles is computed on
DVE/Pool with a bit-trick exp2 (magic-constant round, quadratic 2^f,
exponent-field add) to keep ACT below the PE roofline.
"""

import numpy as np

EMBED = 768
HEADS = 12
HD = 64
SCALE = HD ** -0.5
LOG2E = 1.4426950408889634
LN2 = 0.6931471805599453
MAGIC = 12582912.0  # 1.5 * 2^23
EXPC2, EXPC1, EXPC0 = 0.23842249585793798, -0.7034364107920545, 1.000442696284017
B, T = 4, 2048
NCORES = 8
HPC = 6            # heads per core
DL = HPC * HD      # 384 local model dims per core

NDT = EMBED // 128   # 6 contraction tiles over embed dim
NKT = T // 128       # 16 key-position tiles
NQT = T // 128       # 16 q tiles
QH = 2               # q halves of 1024
QHW = T // QH        # 1024
QT = QHW // 128      # 8 q-tiles per half
NST = QH * NKT       # 32 steps per head
NSTEP = HPC * NST    # 192 global steps

# global steps on which the exp tile is computed on DVE/Pool instead of ACT
DVE_EXP_STEPS = frozenset()

_prog_cache = {}


def _schedule(items, budgets):
    """EDF-greedy: place each (release, deadline, cyc, fn[, after]) item
    in its window at the least-loaded step (relative to that step's
    budget), earliest on ties, processing items in deadline order.
    `after` names an item index whose placed step is a floor for this
    item (emission-order dependency)."""
    nsteps = len(budgets)
    per_step = [[] for _ in range(nsteps)]
    load = [0] * nsteps
    placed = {}
    order = sorted(enumerate(items), key=lambda p: (p[1][1], p[1][0], p[0]))
    progress = True
    while order and progress:
        progress = False
        rest = []
        for idx, it in order:
            release, deadline, cyc, fn = it[:4]
            after = it[4] if len(it) > 4 else None
            if after is not None and after not in placed:
                rest.append((idx, it))
                continue
            if after is not None:
                release = max(release, placed[after])
            deadline = min(deadline, nsteps - 1)
            release = min(release, deadline)
            best = min(range(release, deadline + 1),
                       key=lambda s: (load[s] - budgets[s], s))
            per_step[best].append(fn)
            load[best] += cyc
            placed[idx] = best
            progress = True
        order = rest
    assert not order, "unplaceable items (circular after?)"
    return per_step


def _build_program(repeat=1):
    import concourse.bass as bass
    import concourse.mybir as mybir
    import concourse.tile as tile
    from concourse import bacc

    f32 = mybir.dt.float32
    f32r = mybir.dt.float32r
    bf16 = mybir.dt.bfloat16
    i32 = mybir.dt.int32
    i16 = mybir.dt.int16
    ACT_EXP = mybir.ActivationFunctionType.Exp
    ACT_COPY = mybir.ActivationFunctionType.Copy
    QSC = float(SCALE * LOG2E)

    nc = bacc.Bacc()

    xt_d = nc.dram_tensor("xt", [EMBED, T], f32r, kind="ExternalInput")
    wq_d = nc.dram_tensor("wq", [EMBED, DL], f32r, kind="ExternalInput")
    wk_d = nc.dram_tensor("wk", [EMBED, DL], f32r, kind="ExternalInput")
    wv_d = nc.dram_tensor("wv", [EMBED, DL], bf16, kind="ExternalInput")
    bqs_d = nc.dram_tensor("bqs", [DL], f32, kind="ExternalInput")
    bk_d = nc.dram_tensor("bk", [DL], f32, kind="ExternalInput")
    wp_d = nc.dram_tensor("wp", [DL, EMBED], bf16, kind="ExternalInput")
    aux_d = nc.dram_tensor("aux", [128, 129], f32, kind="ExternalInput")
    y_d = nc.dram_tensor("y", [T, EMBED], bf16, kind="ExternalOutput")
    y2_d = nc.dram_tensor("y2", [T, EMBED], bf16, kind="ExternalOutput")

    with tile.TileContext(nc) as tc:
      for _rep in range(repeat):
        with tc.tile_pool(name="pers", bufs=1) as pers, \
             tc.tile_pool(name="qk", bufs=3) as qk_pool, \
             tc.tile_pool(name="pT", bufs=8) as pT_pool, \
             tc.tile_pool(name="rcp", bufs=2) as rcp_pool, \
             tc.tile_pool(name="ysh", bufs=4) as ysh_pool, \
             tc.tile_pool(name="pss", bufs=2, space="PSUM") as pss_pool, \
             tc.tile_pool(name="po", bufs=1, space="PSUM") as po_pool, \
             tc.tile_pool(name="pd", bufs=1, space="PSUM") as pd_pool, \
             tc.tile_pool(name="ps", bufs=2, space="PSUM") as ps_pool:
            xt_sb = pers.tile([128, 4, NDT, 512], f32r, name="xt_sb")
            xtb_sb = pers.tile([128, 4, NDT, 512], bf16, name="xtb_sb")
            wq_sb = pers.tile([128, NDT, DL], f32r, name="wq_sb")
            wk_sb = pers.tile([128, NDT, DL], f32r, name="wk_sb")
            wv_sb = pers.tile([128, NDT, DL], bf16, name="wv_sb")
            wp_sb = pers.tile([128, 3, EMBED], bf16, name="wp_sb")
            v_sb = pers.tile([128, NKT, HPC, HD], bf16, name="v_sb")
            oT_sb = pers.tile([128, 3, T], bf16, name="oT_sb")
            pack_sb = pers.tile([128, NQT, 128], bf16, name="pack_sb")
            eye_sb = pers.tile([128, 128], bf16, name="eye_sb")
            ones_sb = pers.tile([128, 1], bf16, name="ones_sb")
            auxf_sb = pers.tile([128, 129], f32, name="auxf_sb")
            bqs_sb = pers.tile([128, 3], f32, name="bqs_sb")
            bk_sb = pers.tile([128, 3], f32, name="bk_sb")
            warm_sb = pers.tile([128, 512], f32r, name="warm_sb")

            po_ps = po_pool.tile([128, QT, HD], f32, name="po_ps")
            pd_ps = pd_pool.tile([128, QH, QT], f32, name="pd_ps")

            # ---------------- startup DMAs ----------------
            wq_r = wq_d.ap().rearrange("(n p) m -> p n m", p=128)
            wk_r = wk_d.ap().rearrange("(n p) m -> p n m", p=128)
            wv_r = wv_d.ap().rearrange("(n p) m -> p n m", p=128)

            def dma_xt(ch):
                for dh in range(2):
                    nc.sync.dma_start(
                        out=xt_sb[:, ch, 3 * dh:3 * dh + 3, :],
                        in_=xt_d.ap()[bass.ds(384 * dh, 384), bass.ts(ch, 512)]
                        .rearrange("(n p) m -> p n m", p=128),
                    )

            def copy_xtb(ch):
                for dh in range(2):
                    nc.gpsimd.tensor_copy(
                        out=xtb_sb[:, ch, 3 * dh:3 * dh + 3, :],
                        in_=xt_sb[:, ch, 3 * dh:3 * dh + 3, :],
                    )

            # startup-critical order on the SP queue: x chunk 0 first, then
            # the pair-0 weight columns, chunk 1, wv, the rest.  Tiny
            # tensors ride the Pool DGE queue so they don't spend SP
            # dispatch slots; wp is triggered mid-stream as a filler.
            nc.gpsimd.dma_start(out=bqs_sb, in_=bqs_d.ap().rearrange("(n p) -> p n", p=128))
            nc.gpsimd.dma_start(out=bk_sb, in_=bk_d.ap().rearrange("(n p) -> p n", p=128))
            nc.gpsimd.dma_start(out=auxf_sb, in_=aux_d.ap())
            dma_xt(0)
            nc.sync.dma_start(out=wq_sb[:, :, 0:128], in_=wq_r[:, :, 0:128])
            nc.sync.dma_start(out=wk_sb[:, :, 0:128], in_=wk_r[:, :, 0:128])
            dma_xt(1)
            nc.sync.dma_start(out=wv_sb, in_=wv_r)
            dma_xt(2)
            dma_xt(3)
            nc.sync.dma_start(out=wq_sb[:, :, 128:384], in_=wq_r[:, :, 128:384])
            nc.sync.dma_start(out=wk_sb[:, :, 128:384], in_=wk_r[:, :, 128:384])

            # PE warm-up while the first input DMAs stream (p-state ramp)
            nc.vector.memset(warm_sb.bitcast(f32), 0.0)
            for _wi in range(16):
                psw = ps_pool.tile([128, 512], f32, name="psw", tag="ps")
                nc.tensor.matmul(psw, warm_sb[0:2, 0:128], warm_sb[0:2, :],
                                 start=True, stop=True)

            copy_xtb(0)
            copy_xtb(1)
            nc.vector.tensor_copy(out=eye_sb, in_=auxf_sb[:, 0:128])
            nc.vector.tensor_copy(out=ones_sb, in_=auxf_sb[:, 128:129])

            # ---------------- emitters ----------------
            qk_tiles = {}

            def new_qk(hp):
                if hp in qk_tiles:
                    return
                qTp = qk_pool.tile([128, T], f32r, name="qTp", tag="qT")
                kTp = qk_pool.tile([128, T], f32r, name="kTp", tag="kT")
                qk_tiles[hp] = (qTp, kTp)

            def emit_qk(hp, isq, ch, half):
                """One contraction-half of a qk projection chunk: 3 matmuls
                into a fresh ps tile; half 0 writes qT/kT (with bias), half 1
                accumulates on top."""
                new_qk(hp)
                qTp, kTp = qk_tiles[hp]
                ps = ps_pool.tile([128, 512], f32, name="psqk", tag="ps")
                w_sb = wq_sb if isq else wk_sb
                for dt in range(3 * half, 3 * half + 3):
                    nc.tensor.matmul(
                        ps,
                        w_sb[:, dt, bass.ts(hp, 128)],
                        xt_sb[:, ch, dt, :],
                        start=(dt == 3 * half),
                        stop=(dt == 3 * half + 2),
                    )
                csl = bass.ts(ch, 512)
                if half == 0:
                    if isq:
                        nc.vector.tensor_scalar(
                            out=qTp[:, csl], in0=ps,
                            scalar1=bqs_sb[:, hp:hp + 1],
                            scalar2=QSC,
                            op0=mybir.AluOpType.add,
                            op1=mybir.AluOpType.mult,
                        )
                    else:
                        nc.vector.tensor_scalar_add(
                            out=kTp[:, csl], in0=ps,
                            scalar1=bk_sb[:, hp:hp + 1],
                        )
                else:
                    if isq:
                        nc.vector.scalar_tensor_tensor(
                            out=qTp[:, csl], in0=ps,
                            scalar=QSC, in1=qTp[:, csl],
                            op0=mybir.AluOpType.mult,
                            op1=mybir.AluOpType.add,
                        )
                    else:
                        nc.vector.tensor_add(
                            out=kTp[:, csl], in0=kTp[:, csl], in1=ps,
                        )

            def emit_v_half(kt, half):
                psv = ps_pool.tile([128, DL // 2], f32, name="psv", tag="ps")
                vsl = bass.ds(half * (DL // 2), DL // 2)
                for dt in range(NDT):
                    nc.tensor.matmul(
                        psv,
                        xtb_sb[:, kt // 4, dt, bass.ds((kt % 4) * 128, 128)],
                        wv_sb[:, dt, vsl],
                        start=(dt == 0), stop=(dt == NDT - 1),
                    )
                nc.vector.tensor_copy(
                    out=v_sb[:, kt, 3 * half:3 * half + 3, :],
                    in_=psv.rearrange("p (h d) -> p h d", h=3),
                )

            def transpose_qt(hp, qtg, evac="pool"):
                oTps = ps_pool.tile([128, 128], bf16, name="oTps", tag="ps")
                nc.tensor.matmul(
                    oTps, pack_sb[:, qtg, :], eye_sb, is_transpose=True,
                )
                dst = oT_sb[:, hp, bass.ts(qtg, 128)]
                if evac == "act":
                    nc.scalar.copy(out=dst, in_=oTps)
                else:
                    nc.vector.tensor_copy(out=dst, in_=oTps)

            # ---------------- attention steps ----------------
            SEQ = [(h, qh, kt) for h in range(HPC) for qh in range(QH)
                   for kt in range(NKT)]

            pend_S = {}

            def emit_S_half(h, qh, kt, c2):
                hp, off = h // 2, (h % 2) * 64
                qTp, kTp = qk_tiles[hp]
                if (h, qh, kt) not in pend_S:
                    pend_S[(h, qh, kt)] = pss_pool.tile(
                        [128, QHW], f32, name="pss", tag="pss")
                pss = pend_S[(h, qh, kt)]
                nc.tensor.matmul(
                    pss[:, bass.ts(c2, 512)],
                    kTp[off:off + 64, bass.ts(kt, 128)],
                    qTp[off:off + 64, bass.ds(qh * QHW + c2 * 512, 512)],
                    start=True, stop=True,
                )

            def emit_S(h, qh, kt):
                for c2 in range(QHW // 512):
                    emit_S_half(h, qh, kt, c2)

            def emit_exp(h, qh, kt, split=False):
                """exp2 of a pending S tile on ACT (scale=ln2 folds the
                base-2 pre-scaling back into e^x).  split=True emits two
                512-wide halves so the first can run before the second
                S half-matmul's inputs have even arrived (startup)."""
                pss = pend_S.pop((h, qh, kt))
                pT = pT_pool.tile([128, QHW], bf16, name="pT", tag="pT")
                if split:
                    for c2 in range(2):
                        nc.scalar.activation(
                            out=pT[:, bass.ts(c2, 512)],
                            in_=pss[:, bass.ts(c2, 512)],
                            func=ACT_EXP, scale=LN2)
                else:
                    nc.scalar.activation(out=pT, in_=pss, func=ACT_EXP,
                                         scale=LN2)
                return pT

            def emit_av(h, qh, kt, pT):
                for qt in range(QT):
                    nc.tensor.matmul(
                        pd_ps[:, qh, qt:qt + 1],
                        pT[:, bass.ts(qt, 128)],
                        ones_sb,
                        start=(kt == 0 and qt == 0),
                        stop=(kt == NKT - 1 and qt == QT - 1),
                    )
                    nc.tensor.matmul(
                        po_ps[:, qt, :],
                        pT[:, bass.ts(qt, 128)],
                        v_sb[:, kt, h, :],
                        start=(kt == 0 and qt == 0),
                        stop=(kt == NKT - 1 and qt == QT - 1),
                    )

            def normalize_qh(h, qh, final=False):
                """softmax normalization into the pack tile + the full-range
                memsets that order the next q-half's start=True matmuls
                behind every normalize read."""
                off2 = (h % 2) * 64
                rc = rcp_pool.tile([128, QT], f32, name="rc", tag="rc")
                nc.vector.reciprocal(out=rc, in_=pd_ps[:, qh, :])
                for qt in range(QT):
                    dst = pack_sb[:, qh * QT + qt, off2:off2 + 64]
                    if final and qt % 2 == 0:
                        nc.scalar.activation(
                            out=dst, in_=po_ps[:, qt, :],
                            func=ACT_COPY, scale=rc[:, qt:qt + 1],
                        )
                    else:
                        nc.vector.tensor_scalar_mul(
                            out=dst, in0=po_ps[:, qt, :],
                            scalar1=rc[:, qt:qt + 1],
                        )
                if not final:
                    nc.vector.memset(po_ps[:, :, :], 0.0)
                    nc.vector.memset(pd_ps[:, :, :], 0.0)

            # ---------------- output projection ----------------
            ysh01 = {}

            def proj01_nh(qtg, nh):
                if qtg not in ysh01:
                    ysh01[qtg] = ysh_pool.tile(
                        [128, EMBED], bf16, name="ysh", tag="ysh")
                ysh = ysh01[qtg]
                psy = ps_pool.tile([128, 384], f32, name="psy", tag="ps")
                for dt in range(2):
                    nc.tensor.matmul(
                        psy,
                        oT_sb[:, dt, bass.ts(qtg, 128)],
                        wp_sb[:, dt, bass.ts(nh, 384)],
                        start=(dt == 0), stop=(dt == 1),
                    )
                nc.vector.tensor_copy(out=ysh[:, bass.ts(nh, 384)], in_=psy)
                if nh == 1:
                    nc.sync.dma_start(
                        out=y_d.ap()[bass.ts(qtg, 128), :],
                        in_=ysh01.pop(qtg),
                    )

            def ship2(qtg, deep=False):
                """dt=2 projection partial.  deep=True (tail) uses one dead
                pss tile for both output halves (one per PSUM bank) and
                splits the evacuations DVE/ACT + the DMA across queues."""
                ysh = ysh_pool.tile([128, EMBED], bf16, name="ysh2", tag="ysh")
                if deep:
                    pt = pss_pool.tile([128, QHW], f32, name="psy2d",
                                       tag="pss")
                    slots = (pt[:, 0:384], pt[:, 512:896])
                else:
                    slots = tuple(
                        ps_pool.tile([128, 384], f32, name="psy2", tag="ps")
                        for _ in range(2))
                for nh in range(2):
                    nc.tensor.matmul(
                        slots[nh],
                        oT_sb[:, 2, bass.ts(qtg, 128)],
                        wp_sb[:, 2, bass.ts(nh, 384)],
                        start=True, stop=True,
                    )
                for nh in range(2):
                    dst = ysh[:, bass.ts(nh, 384)]
                    if deep and nh == 1:
                        nc.scalar.copy(out=dst, in_=slots[nh])
                    else:
                        nc.vector.tensor_copy(out=dst, in_=slots[nh])
                eng = nc.scalar if (deep and qtg % 2) else nc.sync
                eng.dma_start(out=y2_d.ap()[bass.ts(qtg, 128), :], in_=ysh)

            # ---------------- filler schedule ----------------
            budgets = [820] * NSTEP
            items = []
            v_emitted = set()

            def add(release, deadline, cyc, fn, after=None):
                items.append((release, deadline, cyc, fn, after))
                return len(items) - 1

            def v_fn(kt, half):
                def f():
                    emit_v_half(kt, half)
                    v_emitted.add((kt, half))
                return f

            qk_cyc, v_cyc, tr_cyc, p01_cyc, s2_cyc = 1536, 1152, 128, 768, 768

            # V half-0 tiles.  wv lands ~mid-B0, the AV deferral absorbs
            # the latency for small kt; deadline kt+4 keeps the pT pool
            # (8 deep) from backing up.
            for kt in range(NKT):
                rel = 4 if kt < 8 else 7 if kt < 12 else 11
                add(rel, min(kt + 4, 15), v_cyc, v_fn(kt, 0))
            # xtb shadow copies for chunks 2,3 (Pool; zero PE cost)
            add(3, 4, 0, lambda: copy_xtb(2))
            add(7, 8, 0, lambda: copy_xtb(3))
            # pair-0 remaining q/k chunks.  S(0,0,kt) is emitted at step
            # kt-2, so k-chunk c is needed by step 4c-3; q chunks 2,3 feed
            # q-half 1 whose first S is emitted at step 14.
            for isq in range(2):
                i0 = add(4, 5 if not isq else 13, qk_cyc,
                         lambda q=isq: emit_qk(0, q, 2, 0))
                add(4, 5 if not isq else 13, qk_cyc,
                    lambda q=isq: emit_qk(0, q, 2, 1), after=i0)
                i1 = add(8, 9 if not isq else 13, qk_cyc,
                         lambda q=isq: emit_qk(0, q, 3, 0))
                add(8, 9 if not isq else 13, qk_cyc,
                    lambda q=isq: emit_qk(0, q, 3, 1), after=i1)
            # pair-1 / pair-2 qk (heads 2,3 start at step 64; 4,5 at 128)
            for hp, base, rel in ((1, 64, 14), (2, 128, 44)):
                for isq in range(2):
                    for ch in range(4):
                        dl = (base - 3 + 4 * ch) if not isq else \
                             (base - 3 if ch < 2 else base + 12)
                        i0 = add(rel, dl, qk_cyc,
                                 lambda p=hp, q=isq, c=ch: emit_qk(p, q, c, 0))
                        add(rel, dl, qk_cyc,
                            lambda p=hp, q=isq, c=ch: emit_qk(p, q, c, 1),
                            after=i0)
            # V half-1 tiles (first consumer: head 3 at step 96)
            for kt in range(NKT):
                add(16, 94 + kt, v_cyc, v_fn(kt, 1))
            # wp weights DMA trigger (Pool DGE queue; zero PE cost)
            add(40, 44, 0, lambda: nc.gpsimd.dma_start(
                out=wp_sb, in_=wp_d.ap().rearrange("(n p) m -> p n m", p=128)))
            # pair-0 transposes: pack ready after head 1 (step 63)
            for qtg in range(NQT):
                add(65, 120, tr_cyc, lambda q=qtg: transpose_qt(0, q))
            # pair-1 transposes: pack ready after head 3 (step 127)
            tr1 = {}
            for qtg in range(NQT):
                tr1[qtg] = add(129, 131 + qtg, tr_cyc,
                               lambda q=qtg: transpose_qt(1, q))
            # output projection dt<2 partials (need pair-0/1 transposes + wp)
            for qtg in range(NQT):
                i0 = add(131, 180, p01_cyc,
                         lambda q=qtg: proj01_nh(q, 0), after=tr1[qtg])
                add(131, 180, p01_cyc,
                    lambda q=qtg: proj01_nh(q, 1), after=i0)
            # pair-2 transposes + dt=2 ships for q-half 0 (pack(qh0) ready
            # after head-5 qh0 = step 175); coupled so the ship's oT read is
            # always emitted after its transpose
            def tr_ship2(qtg):
                transpose_qt(2, qtg)
                ship2(qtg)

            for qtg in range(QT):
                add(177, 183 + qtg, tr_cyc + s2_cyc,
                    lambda q=qtg: tr_ship2(q))

            per_step = _schedule(items, budgets)

            # ---------------- pre-stream ----------------
            # x chunk 0 lands first: pair-0 ch0 projections, then the first
            # S half-matmuls (q columns 0-511) so exp can start while ch1 is
            # still in flight; ch1's projections and the second S halves
            # queue up behind their DMAs.
            new_qk(0)
            for isq in (1, 0):
                emit_qk(0, isq, 0, 0)
                emit_qk(0, isq, 0, 1)
            emit_S_half(0, 0, 0, 0)
            emit_S_half(0, 0, 1, 0)
            for isq in (1, 0):
                emit_qk(0, isq, 1, 0)
                emit_qk(0, isq, 1, 1)
            emit_S_half(0, 0, 0, 1)
            emit_S_half(0, 0, 1, 1)

            # ---------------- main stream ----------------
            # AVs are held in a FIFO and flushed (a) at least two steps
            # after their exp, so q-half boundaries never head-block the
            # in-order PE on the normalize/memset chain, and (b) only once
            # their V tile has been emitted (wv arrives mid-B0).
            av_pending = []

            def flush_avs(i, limit):
                n = 0
                while av_pending and n < limit:
                    ei, eh, eqh, ekt, epT = av_pending[0]
                    if i - ei < 2 or (ekt, eh // 3) not in v_emitted:
                        break
                    av_pending.pop(0)
                    emit_av(eh, eqh, ekt, epT)
                    n += 1

            for i, (h, qh, kt) in enumerate(SEQ):
                for fn in per_step[i]:
                    fn()
                pT = emit_exp(h, qh, kt, split=(i < 2))
                av_pending.append((i, h, qh, kt, pT))
                flush_avs(i, 2)
                if kt == NKT - 1:
                    # drain this q-half's AVs, then normalize
                    while av_pending and av_pending[0][1:3] == (h, qh):
                        _, eh, eqh, ekt, epT = av_pending.pop(0)
                        emit_av(eh, eqh, ekt, epT)
                    normalize_qh(h, qh, final=(i == NSTEP - 1))
                # next step's S matmuls (the step after next: its pss slot
                # was freed by this step's exp)
                if i + 2 < NSTEP:
                    nh_, nqh_, nkt_ = SEQ[i + 2]
                    if (nh_ % 2, nqh_, nkt_) == (0, 0, 0) and nh_ // 2 > h // 2:
                        new_qk(nh_ // 2)
                    emit_S(nh_, nqh_, nkt_)

            # ---------------- tail ----------------
            # pair-2 q-half-1 transposes + ships, pipelined one apart
            transpose_qt(2, QT, evac="dve")
            for qtg in range(QT + 1, NQT):
                transpose_qt(2, qtg, evac=("act" if qtg % 2 == 0 else "dve"))
                ship2(qtg - 1, deep=True)
            ship2(NQT - 1, deep=True)

    nc.finalize()
    return nc


def _shard_inputs(x, w_qkv, b_qkv, w_proj):
    import ml_dtypes

    aux = np.zeros((128, 129), dtype=np.float32)
    aux[:, 0:128] = np.eye(128, dtype=np.float32)
    aux[:, 128] = 1.0
    in_maps = []
    for c in range(NCORES):
        b, g = c // 2, c % 2
        sl = slice(DL * g, DL * g + DL)
        in_maps.append({
            "xt": np.ascontiguousarray(x[b].T),
            "wq": np.ascontiguousarray(w_qkv[:, sl]),
            "wk": np.ascontiguousarray(w_qkv[:, EMBED:][:, sl]),
            "wv": np.ascontiguousarray(w_qkv[:, 2 * EMBED:][:, sl]).astype(ml_dtypes.bfloat16),
            "bqs": np.ascontiguousarray(b_qkv[sl]),
            "bk": np.ascontiguousarray(b_qkv[EMBED:][sl]),
            "wp": np.ascontiguousarray(w_proj[sl, :]).astype(ml_dtypes.bfloat16),
            "aux": aux,
        })
    return in_maps


def kernel(x, w_qkv, b_qkv, w_proj, b_proj, _profile=False, _repeat=1):
    from concourse.bass_utils import run_bass_kernel_spmd

    x = np.asarray(x, dtype=np.float32)
    w_qkv = np.asarray(w_qkv, dtype=np.float32)
    b_qkv = np.asarray(b_qkv, dtype=np.float32)
    w_proj = np.asarray(w_proj, dtype=np.float32)
    b_proj = np.asarray(b_proj, dtype=np.float32)

    if _repeat not in _prog_cache:
        _prog_cache[_repeat] = _build_program(_repeat)
    nc = _prog_cache[_repeat]

    in_maps = _shard_inputs(x, w_qkv, b_qkv, w_proj)
    res = run_bass_kernel_spmd(
        nc, in_maps, list(range(NCORES)), trace=_profile,
    )

    # host-side gather: sum the dt<2 / dt=2 partials of the two head-group
    # cores per batch and add the bias row (v-bias folded through w_proj,
    # plus b_proj itself)
    bias_row = b_qkv[2 * EMBED:] @ w_proj + b_proj
    y = np.empty((B, T, EMBED), dtype=np.float32)
    for b in range(B):
        acc = np.broadcast_to(bias_row.astype(np.float32), (T, EMBED)).copy()
        for c in (2 * b, 2 * b + 1):
            acc += np.asarray(res.results[c]["y"], dtype=np.float32)
            acc += np.asarray(res.results[c]["y2"], dtype=np.float32)
        y[b] = acc
    if _profile:
        return y, res
    return y


# revision 23
# speedup vs baseline: 1.0082x; 1.0021x over previous
"""Multi-head attention (B=4, T=2048, D=768, H=12) on 8 NeuronCores.

Sharding: core c handles batch b = c//2 and head-group g = c%2 (heads
6g..6g+5).  Each core computes its 6 heads' attention and a partial
output projection (contraction over its 384 local dims of w_proj).  The
host sums the two partials per batch and adds the bias terms.

Device-side formulation (per core):
  xT   [768, 2048]  (host pre-transposes x[b])
  qT   = Wq_loc.T @ xT   [384, 2048]   scaled by log2e/sqrt(hd), +bias
  kT   = Wk_loc.T @ xT   [384, 2048]   (+bias)
  v    = x @ Wv_loc      [2048, 384]   (bf16; v-bias folded on host)
  S^T  = kT_h.T @ qT_h   [kpos, q] per (head, kt-tile), base-2 domain
  P^T  = exp2(S^T) = Exp activation with scale=ln2 (bf16)
  O    accumulated in PSUM [q, d] via stationary-P matmuls; softmax
       denominators via ones-column matmuls into a second PSUM bank
  oT   = PE-transpose of the normalized O (bf16, via identity matmul)
  y    = O_loc @ Wp_loc partials shipped bf16 as two tensors; the host
         sums partials from both cores per batch + biases

Schedule: a single global stream of 192 steps (6 heads x 2 q-halves x
16 kt tiles), paced by the ACT exp stream (~1us/step).  Per step the PE
runs the next step's S matmuls, this step's AV/denominator matmuls, and
"filler" work (qkv projections, V tiles, transposes, output projection)
assigned by a small deadline-driven scheduler so PE load stays under
the exp pace everywhere.  At q-half boundaries the AV emissions are
deferred two steps so the in-order PE never head-blocks on the
normalize/memset chain; the exp stream free-runs.  Evacuations are
spread across DVE and Pool.  A subset of exp tiles is computed on
DVE/Pool with a bit-trick exp2 (magic-constant round, quadratic 2^f,
exponent-field add) to keep ACT below the PE roofline.
"""

import numpy as np

EMBED = 768
HEADS = 12
HD = 64
SCALE = HD ** -0.5
LOG2E = 1.4426950408889634
LN2 = 0.6931471805599453
MAGIC = 12582912.0  # 1.5 * 2^23
EXPC2, EXPC1, EXPC0 = 0.23842249585793798, -0.7034364107920545, 1.000442696284017
B, T = 4, 2048
NCORES = 8
HPC = 6            # heads per core
DL = HPC * HD      # 384 local model dims per core

NDT = EMBED // 128   # 6 contraction tiles over embed dim
NKT = T // 128       # 16 key-position tiles
NQT = T // 128       # 16 q tiles
QH = 2               # q halves of 1024
QHW = T // QH        # 1024
QT = QHW // 128      # 8 q-tiles per half
NST = QH * NKT       # 32 steps per head
NSTEP = HPC * NST    # 192 global steps

# global steps on which the exp tile is computed on DVE/Pool instead of ACT
DVE_EXP_STEPS = frozenset()

_prog_cache = {}


def _schedule(items, budgets):
    """EDF-greedy: place each (release, deadline, cyc, fn[, after]) item
    in its window at the least-loaded step (relative to that step's
    budget), earliest on ties, processing items in deadline order.
    `after` names an item index whose placed step is a floor for this
    item (emission-order dependency)."""
    nsteps = len(budgets)
    per_step = [[] for _ in range(nsteps)]
    load = [0] * nsteps
    placed = {}
    order = sorted(enumerate(items), key=lambda p: (p[1][1], p[1][0], p[0]))
    progress = True
    while order and progress:
        progress = False
        rest = []
        for idx, it in order:
            release, deadline, cyc, fn = it[:4]
            after = it[4] if len(it) > 4 else None
            if after is not None and after not in placed:
                rest.append((idx, it))
                continue
            if after is not None:
                release = max(release, placed[after])
            deadline = min(deadline, nsteps - 1)
            release = min(release, deadline)
            best = min(range(release, deadline + 1),
                       key=lambda s: (load[s] - budgets[s], s))
            per_step[best].append(fn)
            load[best] += cyc
            placed[idx] = best
            progress = True
        order = rest
    assert not order, "unplaceable items (circular after?)"
    return per_step


def _build_program(repeat=1):
    import concourse.bass as bass
    import concourse.mybir as mybir
    import concourse.tile as tile
    from concourse import bacc

    f32 = mybir.dt.float32
    f32r = mybir.dt.float32r
    bf16 = mybir.dt.bfloat16
    i32 = mybir.dt.int32
    i16 = mybir.dt.int16
    ACT_EXP = mybir.ActivationFunctionType.Exp
    ACT_COPY = mybir.ActivationFunctionType.Copy
    QSC = float(SCALE * LOG2E)

    nc = bacc.Bacc()

    xt_d = nc.dram_tensor("xt", [EMBED, T], f32r, kind="ExternalInput")
    wq_d = nc.dram_tensor("wq", [EMBED, DL], f32r, kind="ExternalInput")
    wk_d = nc.dram_tensor("wk", [EMBED, DL], f32r, kind="ExternalInput")
    wv_d = nc.dram_tensor("wv", [EMBED, DL], bf16, kind="ExternalInput")
    bqs_d = nc.dram_tensor("bqs", [DL], f32, kind="ExternalInput")
    bk_d = nc.dram_tensor("bk", [DL], f32, kind="ExternalInput")
    wp_d = nc.dram_tensor("wp", [DL, EMBED], bf16, kind="ExternalInput")
    aux_d = nc.dram_tensor("aux", [128, 129], f32, kind="ExternalInput")
    y_d = nc.dram_tensor("y", [T, EMBED], bf16, kind="ExternalOutput")
    y2_d = nc.dram_tensor("y2", [T, EMBED], bf16, kind="ExternalOutput")

    with tile.TileContext(nc) as tc:
      for _rep in range(repeat):
        with tc.tile_pool(name="pers", bufs=1) as pers, \
             tc.tile_pool(name="qk", bufs=3) as qk_pool, \
             tc.tile_pool(name="pT", bufs=8) as pT_pool, \
             tc.tile_pool(name="rcp", bufs=2) as rcp_pool, \
             tc.tile_pool(name="ysh", bufs=4) as ysh_pool, \
             tc.tile_pool(name="pss", bufs=2, space="PSUM") as pss_pool, \
             tc.tile_pool(name="po", bufs=1, space="PSUM") as po_pool, \
             tc.tile_pool(name="pd", bufs=1, space="PSUM") as pd_pool, \
             tc.tile_pool(name="ps", bufs=2, space="PSUM") as ps_pool:
            xt_sb = pers.tile([128, 4, NDT, 512], f32r, name="xt_sb")
            xtb_sb = pers.tile([128, 4, NDT, 512], bf16, name="xtb_sb")
            wq_sb = pers.tile([128, NDT, DL], f32r, name="wq_sb")
            wk_sb = pers.tile([128, NDT, DL], f32r, name="wk_sb")
            wv_sb = pers.tile([128, NDT, DL], bf16, name="wv_sb")
            wp_sb = pers.tile([128, 3, EMBED], bf16, name="wp_sb")
            v_sb = pers.tile([128, NKT, HPC, HD], bf16, name="v_sb")
            oT_sb = pers.tile([128, 3, T], bf16, name="oT_sb")
            pack_sb = pers.tile([128, NQT, 128], bf16, name="pack_sb")
            eye_sb = pers.tile([128, 128], bf16, name="eye_sb")
            ones_sb = pers.tile([128, 1], bf16, name="ones_sb")
            auxf_sb = pers.tile([128, 129], f32, name="auxf_sb")
            bqs_sb = pers.tile([128, 3], f32, name="bqs_sb")
            bk_sb = pers.tile([128, 3], f32, name="bk_sb")
            warm_sb = pers.tile([128, 512], f32r, name="warm_sb")

            po_ps = po_pool.tile([128, QT, HD], f32, name="po_ps")
            pd_ps = pd_pool.tile([128, QH, QT], f32, name="pd_ps")

            # ---------------- startup DMAs ----------------
            wq_r = wq_d.ap().rearrange("(n p) m -> p n m", p=128)
            wk_r = wk_d.ap().rearrange("(n p) m -> p n m", p=128)
            wv_r = wv_d.ap().rearrange("(n p) m -> p n m", p=128)

            def dma_xt(ch):
                for dh in range(2):
                    nc.sync.dma_start(
                        out=xt_sb[:, ch, 3 * dh:3 * dh + 3, :],
                        in_=xt_d.ap()[bass.ds(384 * dh, 384), bass.ts(ch, 512)]
                        .rearrange("(n p) m -> p n m", p=128),
                    )

            def copy_xtb(ch):
                for dh in range(2):
                    nc.gpsimd.tensor_copy(
                        out=xtb_sb[:, ch, 3 * dh:3 * dh + 3, :],
                        in_=xt_sb[:, ch, 3 * dh:3 * dh + 3, :],
                    )

            # startup-critical order on the SP queue: x chunk 0 first, then
            # the pair-0 weight columns, chunk 1, wv, the rest.  Tiny
            # tensors ride the Pool DGE queue so they don't spend SP
            # dispatch slots; wp is triggered mid-stream as a filler.
            nc.gpsimd.dma_start(out=auxf_sb, in_=aux_d.ap())
            nc.gpsimd.dma_start(out=bqs_sb, in_=bqs_d.ap().rearrange("(n p) -> p n", p=128))
            nc.gpsimd.dma_start(out=bk_sb, in_=bk_d.ap().rearrange("(n p) -> p n", p=128))
            dma_xt(0)
            nc.sync.dma_start(out=wq_sb[:, :, 0:128], in_=wq_r[:, :, 0:128])
            nc.sync.dma_start(out=wk_sb[:, :, 0:128], in_=wk_r[:, :, 0:128])
            nc.sync.dma_start(out=wv_sb, in_=wv_r)
            dma_xt(1)
            dma_xt(2)
            dma_xt(3)
            nc.sync.dma_start(out=wq_sb[:, :, 128:384], in_=wq_r[:, :, 128:384])
            nc.sync.dma_start(out=wk_sb[:, :, 128:384], in_=wk_r[:, :, 128:384])
            nc.sync.dma_start(
                out=wp_sb, in_=wp_d.ap().rearrange("(n p) m -> p n m", p=128))

            # PE warm-up while the first input DMAs stream (p-state ramp)
            nc.vector.memset(warm_sb.bitcast(f32), 0.0)
            for _wi in range(16):
                psw = ps_pool.tile([128, 512], f32, name="psw", tag="ps")
                nc.tensor.matmul(psw, warm_sb[0:2, 0:128], warm_sb[0:2, :],
                                 start=True, stop=True)

            # eye/ones staged on Pool so the aux DMA never head-blocks the
            # DVE evacuation queue
            copy_xtb(0)
            nc.gpsimd.tensor_copy(out=eye_sb, in_=auxf_sb[:, 0:128])
            nc.gpsimd.tensor_copy(out=ones_sb, in_=auxf_sb[:, 128:129])
            copy_xtb(1)

            # ---------------- emitters ----------------
            qk_tiles = {}

            def new_qk(hp):
                if hp in qk_tiles:
                    return
                qTp = qk_pool.tile([128, T], f32r, name="qTp", tag="qT")
                kTp = qk_pool.tile([128, T], f32r, name="kTp", tag="kT")
                qk_tiles[hp] = (qTp, kTp)

            def emit_qk(hp, isq, ch, half):
                """One contraction-half of a qk projection chunk: 3 matmuls
                into a fresh ps tile; half 0 writes qT/kT (with bias), half 1
                accumulates on top."""
                new_qk(hp)
                qTp, kTp = qk_tiles[hp]
                ps = ps_pool.tile([128, 512], f32, name="psqk", tag="ps")
                w_sb = wq_sb if isq else wk_sb
                for dt in range(3 * half, 3 * half + 3):
                    nc.tensor.matmul(
                        ps,
                        w_sb[:, dt, bass.ts(hp, 128)],
                        xt_sb[:, ch, dt, :],
                        start=(dt == 3 * half),
                        stop=(dt == 3 * half + 2),
                    )
                csl = bass.ts(ch, 512)
                if half == 0:
                    if isq:
                        nc.vector.tensor_scalar(
                            out=qTp[:, csl], in0=ps,
                            scalar1=bqs_sb[:, hp:hp + 1],
                            scalar2=QSC,
                            op0=mybir.AluOpType.add,
                            op1=mybir.AluOpType.mult,
                        )
                    else:
                        nc.vector.tensor_scalar_add(
                            out=kTp[:, csl], in0=ps,
                            scalar1=bk_sb[:, hp:hp + 1],
                        )
                else:
                    if isq:
                        nc.vector.scalar_tensor_tensor(
                            out=qTp[:, csl], in0=ps,
                            scalar=QSC, in1=qTp[:, csl],
                            op0=mybir.AluOpType.mult,
                            op1=mybir.AluOpType.add,
                        )
                    else:
                        nc.vector.tensor_add(
                            out=kTp[:, csl], in0=kTp[:, csl], in1=ps,
                        )

            def emit_v_half(kt, half):
                psv = ps_pool.tile([128, DL // 2], f32, name="psv", tag="ps")
                vsl = bass.ds(half * (DL // 2), DL // 2)
                for dt in range(NDT):
                    nc.tensor.matmul(
                        psv,
                        xtb_sb[:, kt // 4, dt, bass.ds((kt % 4) * 128, 128)],
                        wv_sb[:, dt, vsl],
                        start=(dt == 0), stop=(dt == NDT - 1),
                    )
                nc.vector.tensor_copy(
                    out=v_sb[:, kt, 3 * half:3 * half + 3, :],
                    in_=psv.rearrange("p (h d) -> p h d", h=3),
                )

            def transpose_qt(hp, qtg, evac="pool"):
                oTps = ps_pool.tile([128, 128], bf16, name="oTps", tag="ps")
                nc.tensor.matmul(
                    oTps, pack_sb[:, qtg, :], eye_sb, is_transpose=True,
                )
                dst = oT_sb[:, hp, bass.ts(qtg, 128)]
                if evac == "act":
                    nc.scalar.copy(out=dst, in_=oTps)
                else:
                    nc.vector.tensor_copy(out=dst, in_=oTps)

            # ---------------- attention steps ----------------
            SEQ = [(h, qh, kt) for h in range(HPC) for qh in range(QH)
                   for kt in range(NKT)]

            pend_S = {}

            def emit_S_half(h, qh, kt, c2):
                hp, off = h // 2, (h % 2) * 64
                qTp, kTp = qk_tiles[hp]
                if (h, qh, kt) not in pend_S:
                    pend_S[(h, qh, kt)] = pss_pool.tile(
                        [128, QHW], f32, name="pss", tag="pss")
                pss = pend_S[(h, qh, kt)]
                nc.tensor.matmul(
                    pss[:, bass.ts(c2, 512)],
                    kTp[off:off + 64, bass.ts(kt, 128)],
                    qTp[off:off + 64, bass.ds(qh * QHW + c2 * 512, 512)],
                    start=True, stop=True,
                )

            def emit_S(h, qh, kt):
                for c2 in range(QHW // 512):
                    emit_S_half(h, qh, kt, c2)

            def emit_exp(h, qh, kt, split=False):
                """exp2 of a pending S tile on ACT (scale=ln2 folds the
                base-2 pre-scaling back into e^x).  split=True emits two
                512-wide halves so the first can run before the second
                S half-matmul's inputs have even arrived (startup)."""
                pss = pend_S.pop((h, qh, kt))
                pT = pT_pool.tile([128, QHW], bf16, name="pT", tag="pT")
                if split:
                    for c2 in range(2):
                        nc.scalar.activation(
                            out=pT[:, bass.ts(c2, 512)],
                            in_=pss[:, bass.ts(c2, 512)],
                            func=ACT_EXP, scale=LN2)
                else:
                    nc.scalar.activation(out=pT, in_=pss, func=ACT_EXP,
                                         scale=LN2)
                return pT

            def emit_av(h, qh, kt, pT):
                for qt in range(QT):
                    nc.tensor.matmul(
                        pd_ps[:, qh, qt:qt + 1],
                        pT[:, bass.ts(qt, 128)],
                        ones_sb,
                        start=(kt == 0 and qt == 0),
                        stop=(kt == NKT - 1 and qt == QT - 1),
                    )
                    nc.tensor.matmul(
                        po_ps[:, qt, :],
                        pT[:, bass.ts(qt, 128)],
                        v_sb[:, kt, h, :],
                        start=(kt == 0 and qt == 0),
                        stop=(kt == NKT - 1 and qt == QT - 1),
                    )

            def normalize_qh(h, qh, final=False):
                """softmax normalization into the pack tile + the full-range
                memsets that order the next q-half's start=True matmuls
                behind every normalize read."""
                off2 = (h % 2) * 64
                rc = rcp_pool.tile([128, QT], f32, name="rc", tag="rc")
                nc.vector.reciprocal(out=rc, in_=pd_ps[:, qh, :])
                for qt in range(QT):
                    dst = pack_sb[:, qh * QT + qt, off2:off2 + 64]
                    if final and qt % 2 == 0:
                        nc.scalar.activation(
                            out=dst, in_=po_ps[:, qt, :],
                            func=ACT_COPY, scale=rc[:, qt:qt + 1],
                        )
                    else:
                        nc.vector.tensor_scalar_mul(
                            out=dst, in0=po_ps[:, qt, :],
                            scalar1=rc[:, qt:qt + 1],
                        )
                if not final:
                    nc.vector.memset(po_ps[:, :, :], 0.0)
                    nc.vector.memset(pd_ps[:, :, :], 0.0)

            # ---------------- output projection ----------------
            ysh01 = {}

            def proj01_nh(qtg, nh):
                if qtg not in ysh01:
                    ysh01[qtg] = ysh_pool.tile(
                        [128, EMBED], bf16, name="ysh", tag="ysh")
                ysh = ysh01[qtg]
                psy = ps_pool.tile([128, 384], f32, name="psy", tag="ps")
                for dt in range(2):
                    nc.tensor.matmul(
                        psy,
                        oT_sb[:, dt, bass.ts(qtg, 128)],
                        wp_sb[:, dt, bass.ts(nh, 384)],
                        start=(dt == 0), stop=(dt == 1),
                    )
                nc.vector.tensor_copy(out=ysh[:, bass.ts(nh, 384)], in_=psy)
                if nh == 1:
                    nc.sync.dma_start(
                        out=y_d.ap()[bass.ts(qtg, 128), :],
                        in_=ysh01.pop(qtg),
                    )

            def ship2(qtg, deep=False):
                """dt=2 projection partial.  deep=True (tail) uses one dead
                pss tile for both output halves (one per PSUM bank) and
                splits the evacuations DVE/ACT + the DMA across queues."""
                ysh = ysh_pool.tile([128, EMBED], bf16, name="ysh2", tag="ysh")
                if deep:
                    pt = pss_pool.tile([128, QHW], f32, name="psy2d",
                                       tag="pss")
                    slots = (pt[:, 0:384], pt[:, 512:896])
                else:
                    slots = tuple(
                        ps_pool.tile([128, 384], f32, name="psy2", tag="ps")
                        for _ in range(2))
                for nh in range(2):
                    nc.tensor.matmul(
                        slots[nh],
                        oT_sb[:, 2, bass.ts(qtg, 128)],
                        wp_sb[:, 2, bass.ts(nh, 384)],
                        start=True, stop=True,
                    )
                for nh in range(2):
                    dst = ysh[:, bass.ts(nh, 384)]
                    if deep and nh == 1:
                        nc.scalar.copy(out=dst, in_=slots[nh])
                    else:
                        nc.vector.tensor_copy(out=dst, in_=slots[nh])
                eng = nc.scalar if (deep and qtg % 2) else nc.sync
                eng.dma_start(out=y2_d.ap()[bass.ts(qtg, 128), :], in_=ysh)

            # ---------------- filler schedule ----------------
            budgets = [820] * NSTEP
            items = []
            v_emitted = set()

            def add(release, deadline, cyc, fn, after=None):
                items.append((release, deadline, cyc, fn, after))
                return len(items) - 1

            def v_fn(kt, half):
                def f():
                    emit_v_half(kt, half)
                    v_emitted.add((kt, half))
                return f

            qk_cyc, v_cyc, tr_cyc, p01_cyc, s2_cyc = 1536, 1152, 128, 768, 768

            # V half-0 tiles.  wv lands ~mid-B0, the AV deferral absorbs
            # the latency for small kt; deadline kt+4 keeps the pT pool
            # (8 deep) from backing up.
            for kt in range(NKT):
                rel = 2 if kt < 4 else 3 if kt < 8 else 7 if kt < 12 else 11
                add(rel, min(kt + 4, 15), v_cyc, v_fn(kt, 0))
            # xtb shadow copies for chunks 2,3 (Pool; zero PE cost)
            add(3, 4, 0, lambda: copy_xtb(2))
            add(7, 8, 0, lambda: copy_xtb(3))
            # pair-0 remaining q/k chunks.  S(0,0,kt) is emitted at step
            # kt-2, so k-chunk c is needed by step 4c-3; q chunks 2,3 feed
            # q-half 1 whose first S is emitted at step 14.
            for isq in range(2):
                i0 = add(4, 5 if not isq else 13, qk_cyc,
                         lambda q=isq: emit_qk(0, q, 2, 0))
                add(4, 5 if not isq else 13, qk_cyc,
                    lambda q=isq: emit_qk(0, q, 2, 1), after=i0)
                i1 = add(8, 9 if not isq else 13, qk_cyc,
                         lambda q=isq: emit_qk(0, q, 3, 0))
                add(8, 9 if not isq else 13, qk_cyc,
                    lambda q=isq: emit_qk(0, q, 3, 1), after=i1)
            # pair-1 / pair-2 qk (heads 2,3 start at step 64; 4,5 at 128)
            for hp, base, rel in ((1, 64, 14), (2, 128, 44)):
                for isq in range(2):
                    for ch in range(4):
                        dl = (base - 3 + 4 * ch) if not isq else \
                             (base - 3 if ch < 2 else base + 12)
                        i0 = add(rel, dl, qk_cyc,
                                 lambda p=hp, q=isq, c=ch: emit_qk(p, q, c, 0))
                        add(rel, dl, qk_cyc,
                            lambda p=hp, q=isq, c=ch: emit_qk(p, q, c, 1),
                            after=i0)
            # V half-1 tiles (first consumer: head 3 at step 96)
            for kt in range(NKT):
                add(16, 94 + kt, v_cyc, v_fn(kt, 1))
            # pair-0 transposes: pack ready after head 1 (step 63)
            for qtg in range(NQT):
                add(65, 120, tr_cyc, lambda q=qtg: transpose_qt(0, q))
            # pair-1 transposes: pack ready after head 3 (step 127)
            tr1 = {}
            for qtg in range(NQT):
                tr1[qtg] = add(129, 131 + qtg, tr_cyc,
                               lambda q=qtg: transpose_qt(1, q))
            # output projection dt<2 partials (need pair-0/1 transposes + wp)
            for qtg in range(NQT):
                i0 = add(131, 180, p01_cyc,
                         lambda q=qtg: proj01_nh(q, 0), after=tr1[qtg])
                add(131, 180, p01_cyc,
                    lambda q=qtg: proj01_nh(q, 1), after=i0)
            # pair-2 transposes + dt=2 ships for q-half 0 (pack(qh0) ready
            # after head-5 qh0 = step 175); coupled so the ship's oT read is
            # always emitted after its transpose
            def tr_ship2(qtg):
                transpose_qt(2, qtg)
                ship2(qtg)

            for qtg in range(QT):
                add(177, 183 + qtg, tr_cyc + s2_cyc,
                    lambda q=qtg: tr_ship2(q))

            per_step = _schedule(items, budgets)

            # ---------------- pre-stream ----------------
            # x chunk 0 lands first: pair-0 ch0 projections, then the first
            # S half-matmuls (q columns 0-511) so exp can start while ch1 is
            # still in flight; ch1's projections and the second S halves
            # queue up behind their DMAs.
            new_qk(0)
            for isq in (1, 0):
                emit_qk(0, isq, 0, 0)
                emit_qk(0, isq, 0, 1)
            emit_S_half(0, 0, 0, 0)
            emit_S_half(0, 0, 1, 0)
            for isq in (1, 0):
                emit_qk(0, isq, 1, 0)
                emit_qk(0, isq, 1, 1)
            emit_S_half(0, 0, 0, 1)
            emit_S_half(0, 0, 1, 1)

            # ---------------- main stream ----------------
            # AVs are held in a FIFO and flushed (a) at least two steps
            # after their exp, so q-half boundaries never head-block the
            # in-order PE on the normalize/memset chain, and (b) only once
            # their V tile has been emitted (wv arrives mid-B0).
            av_pending = []

            def flush_avs(i, limit):
                n = 0
                while av_pending and n < limit:
                    ei, eh, eqh, ekt, epT = av_pending[0]
                    if i - ei < 2 or (ekt, eh // 3) not in v_emitted:
                        break
                    av_pending.pop(0)
                    emit_av(eh, eqh, ekt, epT)
                    n += 1

            for i, (h, qh, kt) in enumerate(SEQ):
                for fn in per_step[i]:
                    fn()
                pT = emit_exp(h, qh, kt, split=(i < 2))
                av_pending.append((i, h, qh, kt, pT))
                flush_avs(i, 2)
                if kt == NKT - 1:
                    # drain this q-half's AVs, then normalize
                    while av_pending and av_pending[0][1:3] == (h, qh):
                        _, eh, eqh, ekt, epT = av_pending.pop(0)
                        emit_av(eh, eqh, ekt, epT)
                    normalize_qh(h, qh, final=(i == NSTEP - 1))
                # next step's S matmuls (the step after next: its pss slot
                # was freed by this step's exp)
                if i + 2 < NSTEP:
                    nh_, nqh_, nkt_ = SEQ[i + 2]
                    if (nh_ % 2, nqh_, nkt_) == (0, 0, 0) and nh_ // 2 > h // 2:
                        new_qk(nh_ // 2)
                    emit_S(nh_, nqh_, nkt_)

            # ---------------- tail ----------------
            # pair-2 q-half-1: all transposes first (evacs split DVE/ACT),
            # then the ships, so the PE never waits mid-chain
            for qtg in range(QT, NQT):
                transpose_qt(2, qtg, evac=("act" if qtg % 2 == 0 else "dve"))
            for qtg in range(QT, NQT):
                ship2(qtg, deep=True)

    nc.finalize()
    return nc


def _shard_inputs(x, w_qkv, b_qkv, w_proj):
    import ml_dtypes

    aux = np.zeros((128, 129), dtype=np.float32)
    aux[:, 0:128] = np.eye(128, dtype=np.float32)
    aux[:, 128] = 1.0
    in_maps = []
    for c in range(NCORES):
        b, g = c // 2, c % 2
        sl = slice(DL * g, DL * g + DL)
        in_maps.append({
            "xt": np.ascontiguousarray(x[b].T),
            "wq": np.ascontiguousarray(w_qkv[:, sl]),
            "wk": np.ascontiguousarray(w_qkv[:, EMBED:][:, sl]),
            "wv": np.ascontiguousarray(w_qkv[:, 2 * EMBED:][:, sl]).astype(ml_dtypes.bfloat16),
            "bqs": np.ascontiguousarray(b_qkv[sl]),
            "bk": np.ascontiguousarray(b_qkv[EMBED:][sl]),
            "wp": np.ascontiguousarray(w_proj[sl, :]).astype(ml_dtypes.bfloat16),
            "aux": aux,
        })
    return in_maps


def kernel(x, w_qkv, b_qkv, w_proj, b_proj, _profile=False, _repeat=1):
    from concourse.bass_utils import run_bass_kernel_spmd

    x = np.asarray(x, dtype=np.float32)
    w_qkv = np.asarray(w_qkv, dtype=np.float32)
    b_qkv = np.asarray(b_qkv, dtype=np.float32)
    w_proj = np.asarray(w_proj, dtype=np.float32)
    b_proj = np.asarray(b_proj, dtype=np.float32)

    if _repeat not in _prog_cache:
        _prog_cache[_repeat] = _build_program(_repeat)
    nc = _prog_cache[_repeat]

    in_maps = _shard_inputs(x, w_qkv, b_qkv, w_proj)
    res = run_bass_kernel_spmd(
        nc, in_maps, list(range(NCORES)), trace=_profile,
    )

    # host-side gather: sum the dt<2 / dt=2 partials of the two head-group
    # cores per batch and add the bias row (v-bias folded through w_proj,
    # plus b_proj itself)
    bias_row = b_qkv[2 * EMBED:] @ w_proj + b_proj
    y = np.empty((B, T, EMBED), dtype=np.float32)
    for b in range(B):
        acc = np.broadcast_to(bias_row.astype(np.float32), (T, EMBED)).copy()
        for c in (2 * b, 2 * b + 1):
            acc += np.asarray(res.results[c]["y"], dtype=np.float32)
            acc += np.asarray(res.results[c]["y2"], dtype=np.float32)
        y[b] = acc
    if _profile:
        return y, res
    return y


# revision 27
# speedup vs baseline: 1.0100x; 1.0018x over previous
"""Multi-head attention (B=4, T=2048, D=768, H=12) on 8 NeuronCores.

Sharding: core c handles batch b = c//2 and head-group g = c%2 (heads
6g..6g+5).  Each core computes its 6 heads' attention and a partial
output projection (contraction over its 384 local dims of w_proj).  The
host sums the two partials per batch and adds the bias terms.

Device-side formulation (per core):
  xT   [768, 2048]  (host pre-transposes x[b])
  qT   = Wq_loc.T @ xT   [384, 2048]   scaled by log2e/sqrt(hd), +bias
  kT   = Wk_loc.T @ xT   [384, 2048]   (+bias)
  v    = x @ Wv_loc      [2048, 384]   (bf16; v-bias folded on host)
  S^T  = kT_h.T @ qT_h   [kpos, q] per (head, kt-tile), base-2 domain
  P^T  = exp2(S^T) = Exp activation with scale=ln2 (bf16)
  O    accumulated in PSUM [q, d] via stationary-P matmuls; softmax
       denominators via ones-column matmuls into a second PSUM bank
  oT   = PE-transpose of the normalized O (bf16, via identity matmul)
  y    = O_loc @ Wp_loc partials shipped bf16 as two tensors; the host
         sums partials from both cores per batch + biases

Schedule: a single global stream of 192 steps (6 heads x 2 q-halves x
16 kt tiles), paced by the ACT exp stream (~1us/step).  Per step the PE
runs the next step's S matmuls, this step's AV/denominator matmuls, and
"filler" work (qkv projections, V tiles, transposes, output projection)
assigned by a small deadline-driven scheduler so PE load stays under
the exp pace everywhere.  At q-half boundaries the AV emissions are
deferred two steps so the in-order PE never head-blocks on the
normalize/memset chain; the exp stream free-runs.  Evacuations are
spread across DVE and Pool.  A subset of exp tiles is computed on
DVE/Pool with a bit-trick exp2 (magic-constant round, quadratic 2^f,
exponent-field add) to keep ACT below the PE roofline.
"""

import numpy as np

EMBED = 768
HEADS = 12
HD = 64
SCALE = HD ** -0.5
LOG2E = 1.4426950408889634
LN2 = 0.6931471805599453
MAGIC = 12582912.0  # 1.5 * 2^23
EXPC2, EXPC1, EXPC0 = 0.23842249585793798, -0.7034364107920545, 1.000442696284017
B, T = 4, 2048
NCORES = 8
HPC = 6            # heads per core
DL = HPC * HD      # 384 local model dims per core

NDT = EMBED // 128   # 6 contraction tiles over embed dim
NKT = T // 128       # 16 key-position tiles
NQT = T // 128       # 16 q tiles
QH = 2               # q halves of 1024
QHW = T // QH        # 1024
QT = QHW // 128      # 8 q-tiles per half
NST = QH * NKT       # 32 steps per head
NSTEP = HPC * NST    # 192 global steps

# global steps on which the exp tile is computed on DVE/Pool instead of ACT
DVE_EXP_STEPS = frozenset()

_prog_cache = {}


def _schedule(items, budgets):
    """EDF-greedy: place each (release, deadline, cyc, fn[, after]) item
    in its window at the least-loaded step (relative to that step's
    budget), earliest on ties, processing items in deadline order.
    `after` names an item index whose placed step is a floor for this
    item (emission-order dependency)."""
    nsteps = len(budgets)
    per_step = [[] for _ in range(nsteps)]
    load = [0] * nsteps
    placed = {}
    order = sorted(enumerate(items), key=lambda p: (p[1][1], p[1][0], p[0]))
    progress = True
    while order and progress:
        progress = False
        rest = []
        for idx, it in order:
            release, deadline, cyc, fn = it[:4]
            after = it[4] if len(it) > 4 else None
            if after is not None and after not in placed:
                rest.append((idx, it))
                continue
            if after is not None:
                release = max(release, placed[after])
            deadline = min(deadline, nsteps - 1)
            release = min(release, deadline)
            best = min(range(release, deadline + 1),
                       key=lambda s: (load[s] - budgets[s], s))
            per_step[best].append(fn)
            load[best] += cyc
            placed[idx] = best
            progress = True
        order = rest
    assert not order, "unplaceable items (circular after?)"
    return per_step


def _build_program(repeat=1):
    import concourse.bass as bass
    import concourse.mybir as mybir
    import concourse.tile as tile
    from concourse import bacc

    f32 = mybir.dt.float32
    f32r = mybir.dt.float32r
    bf16 = mybir.dt.bfloat16
    i32 = mybir.dt.int32
    i16 = mybir.dt.int16
    ACT_EXP = mybir.ActivationFunctionType.Exp
    ACT_COPY = mybir.ActivationFunctionType.Copy
    QSC = float(SCALE * LOG2E)

    nc = bacc.Bacc()

    xt_d = nc.dram_tensor("xt", [EMBED, T], f32r, kind="ExternalInput")
    wq_d = nc.dram_tensor("wq", [EMBED, DL], f32r, kind="ExternalInput")
    wk_d = nc.dram_tensor("wk", [EMBED, DL], f32r, kind="ExternalInput")
    wv_d = nc.dram_tensor("wv", [EMBED, DL], bf16, kind="ExternalInput")
    bqs_d = nc.dram_tensor("bqs", [DL], f32, kind="ExternalInput")
    bk_d = nc.dram_tensor("bk", [DL], f32, kind="ExternalInput")
    wp_d = nc.dram_tensor("wp", [DL, EMBED], bf16, kind="ExternalInput")
    aux_d = nc.dram_tensor("aux", [128, 129], f32, kind="ExternalInput")
    y_d = nc.dram_tensor("y", [T, EMBED], bf16, kind="ExternalOutput")
    y2_d = nc.dram_tensor("y2", [T, EMBED], bf16, kind="ExternalOutput")

    with tile.TileContext(nc) as tc:
      for _rep in range(repeat):
        with tc.tile_pool(name="pers", bufs=1) as pers, \
             tc.tile_pool(name="qk", bufs=3) as qk_pool, \
             tc.tile_pool(name="pT", bufs=8) as pT_pool, \
             tc.tile_pool(name="rcp", bufs=2) as rcp_pool, \
             tc.tile_pool(name="ysh", bufs=4) as ysh_pool, \
             tc.tile_pool(name="pss", bufs=2, space="PSUM") as pss_pool, \
             tc.tile_pool(name="po", bufs=1, space="PSUM") as po_pool, \
             tc.tile_pool(name="pd", bufs=1, space="PSUM") as pd_pool, \
             tc.tile_pool(name="ps", bufs=2, space="PSUM") as ps_pool:
            xt_sb = pers.tile([128, 4, NDT, 512], f32r, name="xt_sb")
            xtb_sb = pers.tile([128, 4, NDT, 512], bf16, name="xtb_sb")
            wq_sb = pers.tile([128, NDT, DL], f32r, name="wq_sb")
            wk_sb = pers.tile([128, NDT, DL], f32r, name="wk_sb")
            wv_sb = pers.tile([128, NDT, DL], bf16, name="wv_sb")
            wp_sb = pers.tile([128, 3, EMBED], bf16, name="wp_sb")
            v_sb = pers.tile([128, NKT, HPC, HD], bf16, name="v_sb")
            oT_sb = pers.tile([128, 3, T], bf16, name="oT_sb")
            pack_sb = pers.tile([128, NQT, 128], bf16, name="pack_sb")
            eye_sb = pers.tile([128, 128], bf16, name="eye_sb")
            ones_sb = pers.tile([128, 1], bf16, name="ones_sb")
            auxf_sb = pers.tile([128, 129], f32, name="auxf_sb")
            bqs_sb = pers.tile([128, 3], f32, name="bqs_sb")
            bk_sb = pers.tile([128, 3], f32, name="bk_sb")
            warm_sb = pers.tile([128, 512], f32r, name="warm_sb")

            po_ps = po_pool.tile([128, QT, HD], f32, name="po_ps")
            pd_ps = pd_pool.tile([128, QH, QT], f32, name="pd_ps")

            # ---------------- startup DMAs ----------------
            wq_r = wq_d.ap().rearrange("(n p) m -> p n m", p=128)
            wk_r = wk_d.ap().rearrange("(n p) m -> p n m", p=128)
            wv_r = wv_d.ap().rearrange("(n p) m -> p n m", p=128)

            def dma_xt(ch):
                for dh in range(2):
                    nc.sync.dma_start(
                        out=xt_sb[:, ch, 3 * dh:3 * dh + 3, :],
                        in_=xt_d.ap()[bass.ds(384 * dh, 384), bass.ts(ch, 512)]
                        .rearrange("(n p) m -> p n m", p=128),
                    )

            def copy_xtb(ch):
                for dh in range(2):
                    nc.gpsimd.tensor_copy(
                        out=xtb_sb[:, ch, 3 * dh:3 * dh + 3, :],
                        in_=xt_sb[:, ch, 3 * dh:3 * dh + 3, :],
                    )

            # startup-critical order on the SP queue: x chunk 0 first, then
            # the pair-0 weight columns, chunk 1, wv, the rest.  Tiny
            # tensors ride the Pool DGE queue so they don't spend SP
            # dispatch slots; wp is triggered mid-stream as a filler.
            nc.gpsimd.dma_start(out=auxf_sb, in_=aux_d.ap())
            nc.gpsimd.dma_start(out=bqs_sb, in_=bqs_d.ap().rearrange("(n p) -> p n", p=128))
            nc.gpsimd.dma_start(out=bk_sb, in_=bk_d.ap().rearrange("(n p) -> p n", p=128))
            dma_xt(0)
            nc.sync.dma_start(out=wq_sb[:, :, 0:128], in_=wq_r[:, :, 0:128])
            nc.sync.dma_start(out=wk_sb[:, :, 0:128], in_=wk_r[:, :, 0:128])
            dma_xt(1)
            nc.sync.dma_start(out=wv_sb, in_=wv_r)
            dma_xt(2)
            dma_xt(3)
            nc.sync.dma_start(out=wq_sb[:, :, 128:384], in_=wq_r[:, :, 128:384])
            nc.sync.dma_start(out=wk_sb[:, :, 128:384], in_=wk_r[:, :, 128:384])
            nc.sync.dma_start(
                out=wp_sb, in_=wp_d.ap().rearrange("(n p) m -> p n m", p=128))

            # PE warm-up while the first input DMAs stream (p-state ramp)
            nc.vector.memset(warm_sb.bitcast(f32), 0.0)
            for _wi in range(16):
                psw = ps_pool.tile([128, 512], f32, name="psw", tag="ps")
                nc.tensor.matmul(psw, warm_sb[0:2, 0:128], warm_sb[0:2, :],
                                 start=True, stop=True)

            # eye/ones staged on Pool so the aux DMA never head-blocks the
            # DVE evacuation queue
            copy_xtb(0)
            nc.gpsimd.tensor_copy(out=eye_sb, in_=auxf_sb[:, 0:128])
            nc.gpsimd.tensor_copy(out=ones_sb, in_=auxf_sb[:, 128:129])
            copy_xtb(1)

            # ---------------- emitters ----------------
            qk_tiles = {}

            def new_qk(hp):
                if hp in qk_tiles:
                    return
                qTp = qk_pool.tile([128, T], f32r, name="qTp", tag="qT")
                kTp = qk_pool.tile([128, T], f32r, name="kTp", tag="kT")
                qk_tiles[hp] = (qTp, kTp)

            def emit_qk(hp, isq, ch, half):
                """One contraction-half of a qk projection chunk: 3 matmuls
                into a fresh ps tile; half 0 writes qT/kT (with bias), half 1
                accumulates on top."""
                new_qk(hp)
                qTp, kTp = qk_tiles[hp]
                ps = ps_pool.tile([128, 512], f32, name="psqk", tag="ps")
                w_sb = wq_sb if isq else wk_sb
                for dt in range(3 * half, 3 * half + 3):
                    nc.tensor.matmul(
                        ps,
                        w_sb[:, dt, bass.ts(hp, 128)],
                        xt_sb[:, ch, dt, :],
                        start=(dt == 3 * half),
                        stop=(dt == 3 * half + 2),
                    )
                csl = bass.ts(ch, 512)
                if half == 0:
                    if isq:
                        nc.vector.tensor_scalar(
                            out=qTp[:, csl], in0=ps,
                            scalar1=bqs_sb[:, hp:hp + 1],
                            scalar2=QSC,
                            op0=mybir.AluOpType.add,
                            op1=mybir.AluOpType.mult,
                        )
                    else:
                        nc.vector.tensor_scalar_add(
                            out=kTp[:, csl], in0=ps,
                            scalar1=bk_sb[:, hp:hp + 1],
                        )
                else:
                    if isq:
                        nc.vector.scalar_tensor_tensor(
                            out=qTp[:, csl], in0=ps,
                            scalar=QSC, in1=qTp[:, csl],
                            op0=mybir.AluOpType.mult,
                            op1=mybir.AluOpType.add,
                        )
                    else:
                        nc.vector.tensor_add(
                            out=kTp[:, csl], in0=kTp[:, csl], in1=ps,
                        )

            def emit_v_half(kt, half):
                psv = ps_pool.tile([128, DL // 2], f32, name="psv", tag="ps")
                vsl = bass.ds(half * (DL // 2), DL // 2)
                for dt in range(NDT):
                    nc.tensor.matmul(
                        psv,
                        xtb_sb[:, kt // 4, dt, bass.ds((kt % 4) * 128, 128)],
                        wv_sb[:, dt, vsl],
                        start=(dt == 0), stop=(dt == NDT - 1),
                    )
                nc.vector.tensor_copy(
                    out=v_sb[:, kt, 3 * half:3 * half + 3, :],
                    in_=psv.rearrange("p (h d) -> p h d", h=3),
                )

            def transpose_qt(hp, qtg, evac="pool"):
                oTps = ps_pool.tile([128, 128], bf16, name="oTps", tag="ps")
                nc.tensor.matmul(
                    oTps, pack_sb[:, qtg, :], eye_sb, is_transpose=True,
                )
                dst = oT_sb[:, hp, bass.ts(qtg, 128)]
                if evac == "act":
                    nc.scalar.copy(out=dst, in_=oTps)
                else:
                    nc.vector.tensor_copy(out=dst, in_=oTps)

            # ---------------- attention steps ----------------
            SEQ = [(h, qh, kt) for h in range(HPC) for qh in range(QH)
                   for kt in range(NKT)]

            pend_S = {}

            def emit_S_half(h, qh, kt, c2):
                """One 512-column S half into its OWN [128,512] pss tile
                (PSUM dependencies are tile-granular, so a shared tile would
                make the first exp wait on the second half's matmul)."""
                hp, off = h // 2, (h % 2) * 64
                qTp, kTp = qk_tiles[hp]
                pss = pss_pool.tile([128, 512], f32, name="pssh", tag="pss")
                nc.tensor.matmul(
                    pss,
                    kTp[off:off + 64, bass.ts(kt, 128)],
                    qTp[off:off + 64, bass.ds(qh * QHW + c2 * 512, 512)],
                    start=True, stop=True,
                )
                pend_S[(h, qh, kt, c2)] = pss

            def emit_S(h, qh, kt):
                qTp, kTp = qk_tiles[h // 2]
                off = (h % 2) * 64
                pss = pss_pool.tile([128, QHW], f32, name="pss", tag="pss")
                for c2 in range(QHW // 512):
                    nc.tensor.matmul(
                        pss[:, bass.ts(c2, 512)],
                        kTp[off:off + 64, bass.ts(kt, 128)],
                        qTp[off:off + 64, bass.ds(qh * QHW + c2 * 512, 512)],
                        start=True, stop=True,
                    )
                pend_S[(h, qh, kt)] = pss

            def emit_exp(h, qh, kt, split=False):
                """exp2 of a pending S tile on ACT (scale=ln2 folds the
                base-2 pre-scaling back into e^x).  split=True emits two
                512-wide halves so the first can run before the second
                S half-matmul's inputs have even arrived (startup)."""
                pss = pend_S.pop((h, qh, kt))
                pT = pT_pool.tile([128, QHW], bf16, name="pT", tag="pT")
                if split:
                    for c2 in range(2):
                        nc.scalar.activation(
                            out=pT[:, bass.ts(c2, 512)],
                            in_=pss[:, bass.ts(c2, 512)],
                            func=ACT_EXP, scale=LN2)
                else:
                    nc.scalar.activation(out=pT, in_=pss, func=ACT_EXP,
                                         scale=LN2)
                return pT

            def emit_av(h, qh, kt, pT):
                for qt in range(QT):
                    nc.tensor.matmul(
                        pd_ps[:, qh, qt:qt + 1],
                        pT[:, bass.ts(qt, 128)],
                        ones_sb,
                        start=(kt == 0 and qt == 0),
                        stop=(kt == NKT - 1 and qt == QT - 1),
                    )
                    nc.tensor.matmul(
                        po_ps[:, qt, :],
                        pT[:, bass.ts(qt, 128)],
                        v_sb[:, kt, h, :],
                        start=(kt == 0 and qt == 0),
                        stop=(kt == NKT - 1 and qt == QT - 1),
                    )

            def normalize_qh(h, qh, final=False):
                """softmax normalization into the pack tile + the full-range
                memsets that order the next q-half's start=True matmuls
                behind every normalize read."""
                off2 = (h % 2) * 64
                rc = rcp_pool.tile([128, QT], f32, name="rc", tag="rc")
                nc.vector.reciprocal(out=rc, in_=pd_ps[:, qh, :])
                for qt in range(QT):
                    dst = pack_sb[:, qh * QT + qt, off2:off2 + 64]
                    if final and qt % 2 == 0:
                        nc.scalar.activation(
                            out=dst, in_=po_ps[:, qt, :],
                            func=ACT_COPY, scale=rc[:, qt:qt + 1],
                        )
                    else:
                        nc.vector.tensor_scalar_mul(
                            out=dst, in0=po_ps[:, qt, :],
                            scalar1=rc[:, qt:qt + 1],
                        )
                if not final:
                    nc.vector.memset(po_ps[:, :, :], 0.0)
                    nc.vector.memset(pd_ps[:, :, :], 0.0)

            # ---------------- output projection ----------------
            ysh01 = {}

            def proj01_nh(qtg, nh):
                if qtg not in ysh01:
                    ysh01[qtg] = ysh_pool.tile(
                        [128, EMBED], bf16, name="ysh", tag="ysh")
                ysh = ysh01[qtg]
                psy = ps_pool.tile([128, 384], f32, name="psy", tag="ps")
                for dt in range(2):
                    nc.tensor.matmul(
                        psy,
                        oT_sb[:, dt, bass.ts(qtg, 128)],
                        wp_sb[:, dt, bass.ts(nh, 384)],
                        start=(dt == 0), stop=(dt == 1),
                    )
                nc.vector.tensor_copy(out=ysh[:, bass.ts(nh, 384)], in_=psy)
                if nh == 1:
                    nc.sync.dma_start(
                        out=y_d.ap()[bass.ts(qtg, 128), :],
                        in_=ysh01.pop(qtg),
                    )

            def ship2(qtg, deep=False):
                """dt=2 projection partial.  deep=True (tail) uses one dead
                pss tile for both output halves (one per PSUM bank) and
                splits the evacuations DVE/ACT + the DMA across queues."""
                ysh = ysh_pool.tile([128, EMBED], bf16, name="ysh2", tag="ysh")
                if deep:
                    pt = pss_pool.tile([128, QHW], f32, name="psy2d",
                                       tag="pss")
                    slots = (pt[:, 0:384], pt[:, 512:896])
                else:
                    slots = tuple(
                        ps_pool.tile([128, 384], f32, name="psy2", tag="ps")
                        for _ in range(2))
                for nh in range(2):
                    nc.tensor.matmul(
                        slots[nh],
                        oT_sb[:, 2, bass.ts(qtg, 128)],
                        wp_sb[:, 2, bass.ts(nh, 384)],
                        start=True, stop=True,
                    )
                for nh in range(2):
                    dst = ysh[:, bass.ts(nh, 384)]
                    if deep and nh == 1:
                        nc.scalar.copy(out=dst, in_=slots[nh])
                    else:
                        nc.vector.tensor_copy(out=dst, in_=slots[nh])
                eng = nc.scalar if (deep and qtg % 2) else nc.sync
                eng.dma_start(out=y2_d.ap()[bass.ts(qtg, 128), :], in_=ysh)

            # ---------------- filler schedule ----------------
            budgets = [820] * NSTEP
            items = []
            v_emitted = set()

            def add(release, deadline, cyc, fn, after=None):
                items.append((release, deadline, cyc, fn, after))
                return len(items) - 1

            def v_fn(kt, half):
                def f():
                    emit_v_half(kt, half)
                    v_emitted.add((kt, half))
                return f

            qk_cyc, v_cyc, tr_cyc, p01_cyc, s2_cyc = 1536, 1152, 128, 768, 768

            # V half-0 tiles.  wv lands ~mid-B0, the AV deferral absorbs
            # the latency for small kt; deadline kt+4 keeps the pT pool
            # (8 deep) from backing up.
            for kt in range(NKT):
                rel = 2 if kt < 4 else 3 if kt < 8 else 7 if kt < 12 else 11
                add(rel, min(kt + 4, 15), v_cyc, v_fn(kt, 0))
            # xtb shadow copies for chunks 2,3 (Pool; zero PE cost)
            add(3, 4, 0, lambda: copy_xtb(2))
            add(7, 8, 0, lambda: copy_xtb(3))
            # pair-0 remaining q/k chunks.  S(0,0,kt) is emitted at step
            # kt-2, so k-chunk c is needed by step 4c-3; q chunks 2,3 feed
            # q-half 1 whose first S is emitted at step 14.
            for isq in range(2):
                i0 = add(4, 5 if not isq else 13, qk_cyc,
                         lambda q=isq: emit_qk(0, q, 2, 0))
                add(4, 5 if not isq else 13, qk_cyc,
                    lambda q=isq: emit_qk(0, q, 2, 1), after=i0)
                i1 = add(8, 9 if not isq else 13, qk_cyc,
                         lambda q=isq: emit_qk(0, q, 3, 0))
                add(8, 9 if not isq else 13, qk_cyc,
                    lambda q=isq: emit_qk(0, q, 3, 1), after=i1)
            # pair-1 / pair-2 qk (heads 2,3 start at step 64; 4,5 at 128)
            for hp, base, rel in ((1, 64, 14), (2, 128, 44)):
                for isq in range(2):
                    for ch in range(4):
                        dl = (base - 3 + 4 * ch) if not isq else \
                             (base - 3 if ch < 2 else base + 12)
                        i0 = add(rel, dl, qk_cyc,
                                 lambda p=hp, q=isq, c=ch: emit_qk(p, q, c, 0))
                        add(rel, dl, qk_cyc,
                            lambda p=hp, q=isq, c=ch: emit_qk(p, q, c, 1),
                            after=i0)
            # V half-1 tiles (first consumer: head 3 at step 96)
            for kt in range(NKT):
                add(16, 94 + kt, v_cyc, v_fn(kt, 1))
            # pair-0 transposes: pack ready after head 1 (step 63)
            for qtg in range(NQT):
                add(65, 120, tr_cyc, lambda q=qtg: transpose_qt(0, q))
            # pair-1 transposes: pack ready after head 3 (step 127)
            tr1 = {}
            for qtg in range(NQT):
                tr1[qtg] = add(129, 131 + qtg, tr_cyc,
                               lambda q=qtg: transpose_qt(1, q))
            # output projection dt<2 partials (need pair-0/1 transposes + wp)
            for qtg in range(NQT):
                i0 = add(131, 180, p01_cyc,
                         lambda q=qtg: proj01_nh(q, 0), after=tr1[qtg])
                add(131, 180, p01_cyc,
                    lambda q=qtg: proj01_nh(q, 1), after=i0)
            # pair-2 transposes + dt=2 ships for q-half 0 (pack(qh0) ready
            # after head-5 qh0 = step 175); coupled so the ship's oT read is
            # always emitted after its transpose
            def tr_ship2(qtg):
                transpose_qt(2, qtg)
                ship2(qtg)

            for qtg in range(QT):
                add(177, 183 + qtg, tr_cyc + s2_cyc,
                    lambda q=qtg: tr_ship2(q))

            per_step = _schedule(items, budgets)

            # ---------------- pre-stream ----------------
            # x chunk 0 lands first: pair-0 ch0 projections, then the first
            # S half-matmuls and their exps (q columns 0-511) so ACT starts
            # while ch1 is still in flight.  ch1's projections are paired by
            # contraction half so work needing only dims 0-383 is not queued
            # behind the second DMA piece.  Each pss slot's exp is emitted
            # before the slot is reused (online WAR tracking).
            new_qk(0)
            for isq in (1, 0):
                emit_qk(0, isq, 0, 0)
                emit_qk(0, isq, 0, 1)
            emit_S_half(0, 0, 0, 0)
            emit_S_half(0, 0, 1, 0)
            pre_pT = [pT_pool.tile([128, QHW], bf16, name="pT", tag="pT")
                      for _ in range(2)]
            for kt in range(2):
                nc.scalar.activation(
                    out=pre_pT[kt][:, 0:512],
                    in_=pend_S.pop((0, 0, kt, 0)),
                    func=ACT_EXP, scale=LN2)
            emit_qk(0, 1, 1, 0)
            emit_qk(0, 0, 1, 0)
            emit_qk(0, 1, 1, 1)
            emit_qk(0, 0, 1, 1)
            emit_S_half(0, 0, 0, 1)
            emit_S_half(0, 0, 1, 1)
            for kt in range(2):
                nc.scalar.activation(
                    out=pre_pT[kt][:, 512:1024],
                    in_=pend_S.pop((0, 0, kt, 1)),
                    func=ACT_EXP, scale=LN2)

            # ---------------- main stream ----------------
            # AVs are held in a FIFO and flushed (a) at least two steps
            # after their exp, so q-half boundaries never head-block the
            # in-order PE on the normalize/memset chain, and (b) only once
            # their V tile has been emitted (wv arrives mid-B0).
            av_pending = []

            def flush_avs(i, limit):
                n = 0
                while av_pending and n < limit:
                    ei, eh, eqh, ekt, epT = av_pending[0]
                    if i - ei < 2 or (ekt, eh // 3) not in v_emitted:
                        break
                    av_pending.pop(0)
                    emit_av(eh, eqh, ekt, epT)
                    n += 1

            for i, (h, qh, kt) in enumerate(SEQ):
                for fn in per_step[i]:
                    fn()
                pT = pre_pT[i] if i < 2 else emit_exp(h, qh, kt)
                av_pending.append((i, h, qh, kt, pT))
                flush_avs(i, 2)
                if kt == NKT - 1:
                    # drain this q-half's AVs, then normalize
                    while av_pending and av_pending[0][1:3] == (h, qh):
                        _, eh, eqh, ekt, epT = av_pending.pop(0)
                        emit_av(eh, eqh, ekt, epT)
                    normalize_qh(h, qh, final=(i == NSTEP - 1))
                # next step's S matmuls (the step after next: its pss slot
                # was freed by this step's exp)
                if i + 2 < NSTEP:
                    nh_, nqh_, nkt_ = SEQ[i + 2]
                    if (nh_ % 2, nqh_, nkt_) == (0, 0, 0) and nh_ // 2 > h // 2:
                        new_qk(nh_ // 2)
                    emit_S(nh_, nqh_, nkt_)

            # ---------------- tail ----------------
            # pair-2 q-half-1: all transposes first (evacs split DVE/ACT),
            # then the ships, so the PE never waits mid-chain
            for qtg in range(QT, NQT):
                transpose_qt(2, qtg, evac=("act" if qtg % 2 == 0 else "dve"))
            for qtg in range(QT, NQT):
                ship2(qtg, deep=True)

    nc.finalize()
    return nc


def _shard_inputs(x, w_qkv, b_qkv, w_proj):
    import ml_dtypes

    aux = np.zeros((128, 129), dtype=np.float32)
    aux[:, 0:128] = np.eye(128, dtype=np.float32)
    aux[:, 128] = 1.0
    in_maps = []
    for c in range(NCORES):
        b, g = c // 2, c % 2
        sl = slice(DL * g, DL * g + DL)
        in_maps.append({
            "xt": np.ascontiguousarray(x[b].T),
            "wq": np.ascontiguousarray(w_qkv[:, sl]),
            "wk": np.ascontiguousarray(w_qkv[:, EMBED:][:, sl]),
            "wv": np.ascontiguousarray(w_qkv[:, 2 * EMBED:][:, sl]).astype(ml_dtypes.bfloat16),
            "bqs": np.ascontiguousarray(b_qkv[sl]),
            "bk": np.ascontiguousarray(b_qkv[EMBED:][sl]),
            "wp": np.ascontiguousarray(w_proj[sl, :]).astype(ml_dtypes.bfloat16),
            "aux": aux,
        })
    return in_maps


def kernel(x, w_qkv, b_qkv, w_proj, b_proj, _profile=False, _repeat=1):
    from concourse.bass_utils import run_bass_kernel_spmd

    x = np.asarray(x, dtype=np.float32)
    w_qkv = np.asarray(w_qkv, dtype=np.float32)
    b_qkv = np.asarray(b_qkv, dtype=np.float32)
    w_proj = np.asarray(w_proj, dtype=np.float32)
    b_proj = np.asarray(b_proj, dtype=np.float32)

    if _repeat not in _prog_cache:
        _prog_cache[_repeat] = _build_program(_repeat)
    nc = _prog_cache[_repeat]

    in_maps = _shard_inputs(x, w_qkv, b_qkv, w_proj)
    res = run_bass_kernel_spmd(
        nc, in_maps, list(range(NCORES)), trace=_profile,
    )

    # host-side gather: sum the dt<2 / dt=2 partials of the two head-group
    # cores per batch and add the bias row (v-bias folded through w_proj,
    # plus b_proj itself)
    bias_row = b_qkv[2 * EMBED:] @ w_proj + b_proj
    y = np.empty((B, T, EMBED), dtype=np.float32)
    for b in range(B):
        acc = np.broadcast_to(bias_row.astype(np.float32), (T, EMBED)).copy()
        for c in (2 * b, 2 * b + 1):
            acc += np.asarray(res.results[c]["y"], dtype=np.float32)
            acc += np.asarray(res.results[c]["y2"], dtype=np.float32)
        y[b] = acc
    if _profile:
        return y, res
    return y
